# revision 1
# baseline (speedup 1.0000x reference)
"""Trainium2 Bass kernel for a 6-layer GPT-style transformer
(B=64, T=256, V=65, D=384, H=6, FF=1536), data-parallel over batch on 8
NeuronCores (8 batch elements = 2048 tokens per core).

Design:
  - Residual stream x lives token-major in SBUF as 16 fp32 tiles [128, 384].
  - LayerNorm in token-major via bn_stats/bn_aggr + fused (x-m)*rstd, with
    ln scale/bias folded into the following matmul weights host-side.
    Output cast to bf16 and DMA-transposed (xbar) to feature-major xnT.
  - Big matmuls run in bf16 (fp32 PSUM accumulate):
      * weights-stationary  -> feature-major outputs (qT, kT, ff1 hidden)
      * activation-stationary -> token-major outputs (v, proj, ff2, head)
  - Biases are applied either as per-partition ACT Identity/Relu bias
    (feature-major) or as K=1 ones-row matmuls accumulated in PSUM
    (token-major).
  - Attention per (batch, head) entirely feature-major with causal
    quadrant skipping; softmax normalization folded into the PSUM->SBUF
    copy of the attention output.
  - Embedding gather = fp32 one-hot matmul (exact); positional add via
    fp32 identity matmul into the same PSUM accumulation.
"""

import os
import numpy as np
import ml_dtypes

import concourse.bass as bass
import concourse.mybir as mybir
import concourse.tile as tile
from concourse.bass_utils import run_bass_kernel_spmd
from contextlib import ExitStack

F32 = mybir.dt.float32
BF16 = mybir.dt.bfloat16
AF = mybir.ActivationFunctionType
OP = mybir.AluOpType

B, T, V, D, H, L = 64, 256, 65, 384, 6, 6
HD = D // H          # 64
FF = 4 * D           # 1536
EPS = 1e-5
SCALE = D ** -0.5

NCORES = 8
BPC = B // NCORES    # 8 batch elements per core
NTOK = BPC * T       # 2048 tokens per core
TT = NTOK // 128     # 16 token tiles
KT = D // 128        # 3 feature tiles
FT = FF // 128       # 12 ff tiles
NCH = 4              # 512-token chunks for feature-major matmuls
FFCH = 2             # ff processed in 2 chunks of 1024 tokens
FTOK = NTOK // FFCH  # 1024


def _split_multi_waits(nc):
    """This walrus build rejects >1 sync wait per instruction; hoist extras
    onto standalone EventSemaphore instructions on the same engine queue."""
    ctr = 0
    for func in nc.m.functions:
        for bb in func.blocks:
            insts = list(bb.instructions)
            out = []
            changed = False
            for inst in insts:
                si = inst.sync_info
                if si is not None and len(si.on_wait) > 1:
                    waits = list(si.on_wait)
                    for w in waits[:-1]:
                        ev = mybir.InstEventSemaphore(
                            name=f"splitwait_{ctr}", ins=[], outs=[])
                        ctr += 1
                        ev.engine = inst.engine
                        ev.sync_info = mybir.SyncInfo(on_wait=[w], on_update=[])
                        nc.register_instruction(ev, overwrite=True)
                        out.append(ev)
                    inst.sync_info = mybir.SyncInfo(
                        on_wait=[waits[-1]], on_update=list(si.on_update))
                    changed = True
                out.append(inst)
            if changed:
                bb.instructions = out


DBG = None  # e.g. ("x0", 16, 128, 384, "f32") stage tag set by tests


def build(n_layers=L):
    nc = bass.Bass(trn_type="TRN2", num_devices=NCORES)

    def din(name, shape, dt):
        return nc.dram_tensor(name, shape, dt, kind="ExternalInput").ap()

    onehotT = din("onehotT", [V, NTOK], F32)
    tokemb = din("tokemb", [V, D], F32)
    pos = din("pos", [T, D], F32)
    ident = din("ident", [128, 128], F32)
    trimask = din("trimask", [128, 128], F32)
    ones_row = din("ones_row", [1, 128], BF16)
    if n_layers:
        wqkv = din("wqkv", [n_layers, 128, KT, 3 * D], BF16)
        bqk = din("bqk", [n_layers, 128, 6], F32)
        bv = din("bv", [n_layers, 1, D], BF16)
        wproj = din("wproj", [n_layers, 128, KT, D], BF16)
        bproj = din("bproj", [n_layers, 1, D], BF16)
        wff1 = din("wff1", [n_layers, 128, KT, FF], BF16)
        bff1 = din("bff1", [n_layers, 128, FT], F32)
        wff2 = din("wff2", [n_layers, 128, FT, D], BF16)
        bff2 = din("bff2", [n_layers, 1, D], BF16)
    whead = din("whead", [128, KT, V], BF16)
    bhead = din("bhead", [1, V], BF16)
    logits = nc.dram_tensor("logits", [NTOK, V], F32, kind="ExternalOutput").ap()
    dbg_spec = {
        "x0": (16, 128, D, F32), "xnT": (KT, 128, NTOK, BF16),
        "qT": (KT, 128, NTOK, BF16), "kT": (KT, 128, NTOK, BF16),
        "vt": (16, 128, D, BF16), "oT": (KT, 128, NTOK, BF16),
        "x1": (16, 128, D, F32), "x2": (16, 128, D, F32),
        "xnT2": (KT, 128, NTOK, BF16),
        "attE": (4, 128, 256, BF16), "attO": (4, 128, 256, F32),
    }
    dbg_ap = None
    if DBG is not None:
        n, p, c, dt = dbg_spec[DBG]
        dbg_ap = nc.dram_tensor("dbg", [n, p, c], dt, kind="ExternalOutput").ap()

    def dump(tag, tiles):
        if DBG == tag:
            for i, tl in enumerate(tiles):
                nc.sync.dma_start(out=dbg_ap[i], in_=tl)

    def dump3(tag, t3):
        if DBG == tag:
            for i in range(KT):
                nc.sync.dma_start(out=dbg_ap[i], in_=t3[:, i, :])

    with tile.TileContext(nc) as tc, ExitStack() as ctx:
        pool = lambda name, bufs: ctx.enter_context(tc.tile_pool(name=name, bufs=bufs))
        const_p = pool("const", 1)
        xres_p = pool("xres", 1)
        xnT_p = pool("xnT", 1)
        qkT_p = pool("qkT", 1)
        v_p = pool("vtok", 1)
        oT_p = pool("oT", 1)
        h_p = pool("hff", 1)
        w_p = pool("wts", 1)
        ln_p = pool("ln", 4)
        at_p = pool("attn", 4)
        cp_p = pool("cpy", 3)

        # ---- constants ----
        tri_s = const_p.tile([128, 128], F32)
        nc.sync.dma_start(out=tri_s, in_=trimask)
        ones_s = const_p.tile([1, 128], BF16)
        nc.sync.dma_start(out=ones_s, in_=ones_row)
        whead_s = const_p.tile([128, KT, V], BF16)
        nc.sync.dma_start(out=whead_s, in_=whead)
        bhead_s = const_p.tile([1, V], BF16)
        nc.sync.dma_start(out=bhead_s, in_=bhead)
        eps_t = const_p.tile([128, 1], F32)
        nc.vector.memset(eps_t, EPS)
        ones128 = const_p.tile([128, 128], BF16)
        nc.vector.memset(ones128, 1.0)

        # ---- persistent activation tiles ----
        x = [xres_p.tile([128, D], F32, tag=f"x{t}", name=f"x{t}") for t in range(TT)]
        xnT = xnT_p.tile([128, KT, NTOK], BF16, tag="xnT", name="xnT")
        qT = [qkT_p.tile([128, NTOK], BF16, tag=f"qT{k}", name=f"qT{k}") for k in range(KT)]
        kT = [qkT_p.tile([128, NTOK], BF16, tag=f"kT{k}", name=f"kT{k}") for k in range(KT)]
        vt = [v_p.tile([128, D], BF16, tag=f"v{t}", name=f"v{t}") for t in range(TT)]
        oT = [oT_p.tile([128, NTOK], BF16, tag=f"oT{k}", name=f"oT{k}") for k in range(KT)]
        hh = [h_p.tile([128, FTOK], BF16, tag=f"h{f}", name=f"h{f}") for f in range(FT)]

        # ---- embedding: x = onehot @ tok_emb + pos ----
        with tc.tile_pool(name="emb_ps", bufs=3, space="PSUM") as emb_ps, \
             tc.tile_pool(name="emb_sb", bufs=1) as emb_sb:
            oh_s = emb_sb.tile([V, NTOK], F32)
            nc.sync.dma_start(out=oh_s, in_=onehotT)
            te_s = emb_sb.tile([V, D], F32)
            nc.sync.dma_start(out=te_s, in_=tokemb)
            pos_s = emb_sb.tile([128, 2, D], F32)
            nc.sync.dma_start(out=pos_s, in_=pos.rearrange("(a p) n -> p a n", p=128))
            id_s = emb_sb.tile([128, 128], F32)
            nc.sync.dma_start(out=id_s, in_=ident)
            for t in range(TT):
                ps = emb_ps.tile([128, D], F32, tag="emb")
                nc.tensor.matmul(ps, lhsT=oh_s[:, t * 128:(t + 1) * 128],
                                 rhs=te_s, start=True, stop=False)
                nc.tensor.matmul(ps, lhsT=id_s, rhs=pos_s[:, t % 2, :],
                                 start=False, stop=True)
                nc.scalar.copy(out=x[t], in_=ps)
        dump("x0", x)

        def layernorm_and_transpose(dst_T):
            """token-major LN over x -> bf16 -> DMA-transpose into dst_T."""
            ctx2 = nc.named_scope("ln")
            ctx2.__enter__()
            for t in range(TT):
                stats = ln_p.tile([128, 6], F32, tag="stats")
                nc.vector.bn_stats(out=stats, in_=x[t])
                mv = ln_p.tile([128, 2], F32, tag="mv")
                nc.vector.bn_aggr(out=mv, in_=stats)
                rstd = ln_p.tile([128, 1], F32, tag="rstd")
                nc.scalar.activation(out=rstd, in_=mv[:, 1:2], func=AF.Sqrt,
                                     bias=eps_t)
                nc.vector.reciprocal(out=rstd, in_=rstd)
                xn16 = ln_p.tile([128, D], BF16, tag="xn16")
                nc.vector.tensor_scalar(out=xn16, in0=x[t], scalar1=mv[:, 0:1],
                                        scalar2=rstd, op0=OP.subtract, op1=OP.mult)
                nc.sync.dma_start(out=dst_T[:, :, t * 128:(t + 1) * 128],
                                  in_=xn16, transpose=True)
            ctx2.__exit__(None, None, None)

        for l in range(n_layers):
            # ---- layer weights ----
            wqkv_s = w_p.tile([128, KT, 3 * D], BF16, tag="wqkv")
            nc.gpsimd.dma_start(out=wqkv_s, in_=wqkv[l])
            bqk_s = w_p.tile([128, 6], F32, tag="bqk")
            nc.gpsimd.dma_start(out=bqk_s, in_=bqk[l])
            bv_s = w_p.tile([1, D], BF16, tag="bv")
            nc.gpsimd.dma_start(out=bv_s, in_=bv[l])
            wproj_s = w_p.tile([128, KT, D], BF16, tag="wproj")
            nc.gpsimd.dma_start(out=wproj_s, in_=wproj[l])
            bproj_s = w_p.tile([1, D], BF16, tag="bproj")
            nc.gpsimd.dma_start(out=bproj_s, in_=bproj[l])
            wff1_s = w_p.tile([128, KT, FF], BF16, tag="wff1")
            nc.gpsimd.dma_start(out=wff1_s, in_=wff1[l])
            bff1_s = w_p.tile([128, FT], F32, tag="bff1")
            nc.gpsimd.dma_start(out=bff1_s, in_=bff1[l])
            wff2_s = w_p.tile([128, FT, D], BF16, tag="wff2")
            nc.gpsimd.dma_start(out=wff2_s, in_=wff2[l])
            bff2_s = w_p.tile([1, D], BF16, tag="bff2")
            nc.gpsimd.dma_start(out=bff2_s, in_=bff2[l])

            # ---- LN1 -> xnT ----
            layernorm_and_transpose(xnT)
            if l == 0:
                dump3("xnT", xnT)

            # ---- qT, kT feature-major ----
            with tc.tile_pool(name="qk_ps", bufs=3, space="PSUM") as qk_ps, \
                 nc.named_scope("qkv"):
                for m in range(6):  # 6 chunks of 128 over q|k (768 cols)
                    dst = qT[m] if m < KT else kT[m - KT]
                    for n in range(NCH):
                        ns = slice(n * 512, (n + 1) * 512)
                        ps = qk_ps.tile([128, 512], F32, tag="qk")
                        for k in range(KT):
                            nc.tensor.matmul(
                                ps, lhsT=wqkv_s[:, k, m * 128:(m + 1) * 128],
                                rhs=xnT[:, k, ns], start=(k == 0), stop=(k == KT - 1))
                        nc.scalar.activation(out=dst[:, ns], in_=ps, func=AF.Identity,
                                             bias=bqk_s[:, m:m + 1])

                # ---- v token-major (same psum pool scope) ----
                for t in range(TT):
                    ps = qk_ps.tile([128, D], F32, tag="vps")
                    for k in range(KT):
                        nc.tensor.matmul(ps, lhsT=xnT[:, k, t * 128:(t + 1) * 128],
                                         rhs=wqkv_s[:, k, 2 * D:3 * D],
                                         start=(k == 0), stop=False)
                    nc.tensor.matmul(ps, lhsT=ones_s,
                                     rhs=bv_s, start=False, stop=True)
                    nc.vector.tensor_copy(out=vt[t], in_=ps)
            if l == 0:
                dump("qT", qT)
                dump("kT", kT)
                dump("vt", vt)

            # ---- attention, feature-major, causal quadrants ----
            with tc.tile_pool(name="sc_ps", bufs=2, space="PSUM") as sc_psp, \
                 tc.tile_pool(name="lo_ps", bufs=2, space="PSUM") as lo_psp, \
                 nc.named_scope("attn"):
                for b in range(BPC):
                    n0 = b * T
                    for h in range(H):
                        hp, r = h // 2, (h % 2) * 64
                        ks0 = kT[hp][r:r + 64, n0:n0 + 128]
                        ks1 = kT[hp][r:r + 64, n0 + 128:n0 + 256]
                        sc0 = sc_psp.tile([128, 256], F32, tag="sc0")
                        sc1 = sc_psp.tile([128, 128], F32, tag="sc1")
                        nc.tensor.matmul(sc0, lhsT=ks0,
                                         rhs=qT[hp][r:r + 64, n0:n0 + 256],
                                         start=True, stop=True)
                        nc.tensor.matmul(sc1, lhsT=ks1,
                                         rhs=qT[hp][r:r + 64, n0 + 128:n0 + 256],
                                         start=True, stop=True)
                        ef0 = at_p.tile([128, 128], F32, tag="ef0")
                        nc.scalar.activation(out=ef0, in_=sc0[:, 0:128],
                                             func=AF.Exp, scale=SCALE)
                        e0 = at_p.tile([128, 256], BF16, tag="e0")
                        nc.gpsimd.tensor_tensor(out=e0[:, 0:128], in0=ef0,
                                                in1=tri_s, op=OP.mult)
                        nc.scalar.activation(out=e0[:, 128:256],
                                             in_=sc0[:, 128:256],
                                             func=AF.Exp, scale=SCALE)
                        ef1 = at_p.tile([128, 128], F32, tag="ef1")
                        nc.scalar.activation(out=ef1, in_=sc1, func=AF.Exp,
                                             scale=SCALE)
                        e1 = at_p.tile([128, 128], BF16, tag="e1")
                        nc.gpsimd.tensor_tensor(out=e1, in0=ef1, in1=tri_s,
                                                op=OP.mult)
                        if DBG == "attE" and l == 0 and b == 7 and hp == 1:
                            hi = h % 2
                            nc.sync.dma_start(out=dbg_ap[2 * hi, :, :], in_=e0)
                            nc.sync.dma_start(out=dbg_ap[2 * hi + 1, :, 0:128],
                                              in_=e1)

                        l_ps = lo_psp.tile([128, 256], F32, tag="lps")
                        nc.tensor.matmul(l_ps, lhsT=ones128, rhs=e0,
                                         start=True, stop=False)
                        nc.tensor.matmul(l_ps[:, 128:256], lhsT=ones128,
                                         rhs=e1, start=False, stop=True)
                        linv = at_p.tile([64, 256], F32, tag="linv")
                        lnl = at_p.tile([64, 256], F32, tag="lnl")
                        nc.scalar.activation(out=lnl, in_=l_ps[0:64, :], func=AF.Ln)
                        nc.scalar.activation(out=linv, in_=lnl, func=AF.Exp,
                                             scale=-1.0)

                        o_ps = lo_psp.tile([64, 256], F32, tag="ops")
                        nc.tensor.matmul(o_ps, lhsT=vt[2 * b][:, h * 64:(h + 1) * 64],
                                         rhs=e0, start=True, stop=False)
                        nc.tensor.matmul(o_ps[:, 128:256],
                                         lhsT=vt[2 * b + 1][:, h * 64:(h + 1) * 64],
                                         rhs=e1, start=False, stop=True)
                        if r == 0:
                            nc.vector.tensor_tensor(
                                out=oT[hp][0:64, n0:n0 + 256],
                                in0=o_ps, in1=linv, op=OP.mult)
                        else:
                            osb = at_p.tile([64, 256], BF16, tag="osb")
                            nc.vector.tensor_tensor(out=osb, in0=o_ps, in1=linv,
                                                    op=OP.mult)
                            nc.gpsimd.dma_start(out=oT[hp][64:128, n0:n0 + 256],
                                                in_=osb)
                if l == 0:
                    dump("oT", oT)

            # ---- proj token-major + residual ----
            with tc.tile_pool(name="tok_ps", bufs=3, space="PSUM") as tok_ps, \
                 nc.named_scope("projff"):
                for t in range(TT):
                    ps = tok_ps.tile([128, D], F32, tag="tok")
                    for k in range(KT):
                        nc.tensor.matmul(ps, lhsT=oT[k][:, t * 128:(t + 1) * 128],
                                         rhs=wproj_s[:, k, :],
                                         start=(k == 0), stop=False)
                    nc.tensor.matmul(ps, lhsT=ones_s,
                                     rhs=bproj_s, start=False, stop=True)
                    nc.vector.tensor_tensor(out=x[t], in0=x[t], in1=ps, op=OP.add)
                if l == 0:
                    dump("x1", x)

                # ---- LN2 -> xnT ----
                layernorm_and_transpose(xnT)
                if l == 0:
                    dump3("xnT2", xnT)

                # ---- FF in two 1024-token chunks ----
                for ch in range(FFCH):
                    c0 = ch * FTOK
                    for f in range(FT):
                        for n in range(FTOK // 512):
                            ns = slice(c0 + n * 512, c0 + (n + 1) * 512)
                            hs = slice(n * 512, (n + 1) * 512)
                            ps = tok_ps.tile([128, 512], F32, tag="ff1")
                            for k in range(KT):
                                nc.tensor.matmul(
                                    ps, lhsT=wff1_s[:, k, f * 128:(f + 1) * 128],
                                    rhs=xnT[:, k, ns], start=(k == 0),
                                    stop=(k == KT - 1))
                            nc.scalar.activation(out=hh[f][:, hs], in_=ps,
                                                 func=AF.Relu,
                                                 bias=bff1_s[:, f:f + 1])
                    for tt in range(FTOK // 128):
                        t = ch * (FTOK // 128) + tt
                        ps = tok_ps.tile([128, D], F32, tag="tok")
                        for f in range(FT):
                            nc.tensor.matmul(
                                ps, lhsT=hh[f][:, tt * 128:(tt + 1) * 128],
                                rhs=wff2_s[:, f, :], start=(f == 0), stop=False)
                        nc.tensor.matmul(ps, lhsT=ones_s,
                                         rhs=bff2_s, start=False, stop=True)
                        nc.vector.tensor_tensor(out=x[t], in0=x[t], in1=ps, op=OP.add)
            if l == 0:
                dump("x2", x)

        # ---- final LN + head ----
        layernorm_and_transpose(xnT)
        with tc.tile_pool(name="hd_ps", bufs=3, space="PSUM") as hd_ps:
            for t in range(TT):
                ps = hd_ps.tile([128, V], F32, tag="hd")
                for k in range(KT):
                    nc.tensor.matmul(ps, lhsT=xnT[:, k, t * 128:(t + 1) * 128],
                                     rhs=whead_s[:, k, :], start=(k == 0), stop=False)
                nc.tensor.matmul(ps, lhsT=ones_s,
                                 rhs=bhead_s, start=False, stop=True)
                lt = cp_p.tile([128, V], F32, tag="logit")
                nc.scalar.copy(out=lt, in_=ps)
                nc.sync.dma_start(out=logits[t * 128:(t + 1) * 128, :], in_=lt)

    _split_multi_waits(nc)
    return nc


def prepare_host_inputs(idx, tok_emb, pos_emb, ln1_w, ln1_b, wq, wk, wv,
                        proj_w, proj_b, ln2_w, ln2_b, ff_w1, ff_b1, ff_w2,
                        ff_b2, lnf_w, lnf_b, head_w, head_b, n_layers=L):
    f32 = np.float32
    bf = ml_dtypes.bfloat16

    def kt_tiles(w, ncols):  # [D, ncols] -> [128, KT, ncols]
        return np.ascontiguousarray(
            w.reshape(KT, 128, ncols).transpose(1, 0, 2))

    wqkv_l, bqk_l, bv_l = [], [], []
    wproj_l, bproj_l = [], []
    wff1_l, bff1_l, wff2_l, bff2_l = [], [], [], []
    for l in range(n_layers):
        # [H, D, HD] -> [D, H*HD]
        q2 = wq[l].transpose(1, 0, 2).reshape(D, D).astype(f32)
        k2 = wk[l].transpose(1, 0, 2).reshape(D, D).astype(f32)
        v2 = wv[l].transpose(1, 0, 2).reshape(D, D).astype(f32)
        qf = ln1_w[l][:, None] * q2
        kf = ln1_w[l][:, None] * k2
        vf = ln1_w[l][:, None] * v2
        bq = ln1_b[l] @ q2
        bk = ln1_b[l] @ k2
        bvv = ln1_b[l] @ v2
        wqkv_l.append(kt_tiles(np.concatenate([qf, kf, vf], axis=1), 3 * D))
        # col m (m<3) = bq[m*128+p]; col 3+m = bk[m*128+p]
        bqk_l.append(np.concatenate(
            [bq.reshape(KT, 128).T, bk.reshape(KT, 128).T], axis=1))
        bv_l.append(bvv.reshape(1, D))
        wproj_l.append(kt_tiles(proj_w[l].astype(f32), D))
        bproj_l.append(proj_b[l].reshape(1, D))
        w1f = ln2_w[l][:, None] * ff_w1[l].astype(f32)
        b1f = ff_b1[l] + ln2_b[l] @ ff_w1[l]
        wff1_l.append(kt_tiles(w1f, FF))
        bff1_l.append(np.ascontiguousarray(
            b1f.reshape(FT, 128).T).astype(f32))
        wff2_l.append(np.ascontiguousarray(
            ff_w2[l].reshape(FT, 128, D).transpose(1, 0, 2)).astype(f32))
        bff2_l.append(ff_b2[l].reshape(1, D))

    whf = lnf_w[:, None] * head_w.astype(f32)
    bhf = head_b + lnf_b @ head_w

    def stk(lst, shape, dt):
        if lst:
            return np.stack(lst).astype(dt)
        return np.zeros((0,) + shape, dt)

    shared = {
        "tokemb": np.asarray(tok_emb, f32),
        "pos": np.asarray(pos_emb, f32),
        "ident": np.eye(128, dtype=f32),
        "trimask": np.triu(np.ones((128, 128), f32)),  # [s,t] valid s<=t
        "ones_row": np.ones((1, 128), bf),
        "wqkv": stk(wqkv_l, (128, KT, 3 * D), bf),
        "bqk": stk(bqk_l, (128, 6), f32),
        "bv": stk(bv_l, (1, D), bf),
        "wproj": stk(wproj_l, (128, KT, D), bf),
        "bproj": stk(bproj_l, (1, D), bf),
        "wff1": stk(wff1_l, (128, KT, FF), bf),
        "bff1": stk(bff1_l, (128, FT), f32),
        "wff2": stk(wff2_l, (128, FT, D), bf),
        "bff2": stk(bff2_l, (1, D), bf),
        "whead": kt_tiles(whf, V).astype(bf),
        "bhead": bhf.reshape(1, V).astype(bf),
    }
    idx = np.asarray(idx)
    in_maps = []
    for c in range(NCORES):
        oh = np.zeros((V, NTOK), f32)
        sl = idx[c * BPC:(c + 1) * BPC].reshape(NTOK)  # token n = b*T + t
        oh[sl, np.arange(NTOK)] = 1.0
        in_maps.append({**shared, "onehotT": oh})
    return in_maps


_CACHED = {}


def kernel(**inputs):
    n_layers = L
    if "nc" not in _CACHED:
        _CACHED["nc"] = build(n_layers)
    nc = _CACHED["nc"]
    in_maps = prepare_host_inputs(**inputs, n_layers=n_layers)
    res = run_bass_kernel_spmd(nc, in_maps, list(range(NCORES)))
    out = np.concatenate(
        [res.results[c]["logits"].reshape(BPC, T, V) for c in range(NCORES)],
        axis=0)
    return out



# revision 13
# speedup vs baseline: 1.2272x; 1.2272x over previous
"""Trainium2 Bass kernel for a 6-layer GPT-style transformer
(B=64, T=256, V=65, D=384, H=6, FF=1536), data-parallel over batch on 8
NeuronCores (8 batch elements = 2048 tokens per core).

Design:
  - Residual stream x lives token-major in SBUF as 16 fp32 tiles [128, 384].
  - LayerNorm in token-major via bn_stats/bn_aggr + fused (x-m)*rstd, with
    ln scale/bias folded into the following matmul weights host-side.
    Output cast to bf16 and DMA-transposed (xbar) to feature-major xnT.
  - Big matmuls run in bf16 (fp32 PSUM accumulate):
      * weights-stationary  -> feature-major outputs (qT, kT, ff1 hidden)
      * activation-stationary -> token-major outputs (v, proj, ff2, head)
  - Biases are applied either as per-partition ACT Identity/Relu bias
    (feature-major) or as K=1 ones-row matmuls accumulated in PSUM
    (token-major).
  - Attention per (batch, head) entirely feature-major with causal
    quadrant skipping; softmax normalization folded into the PSUM->SBUF
    copy of the attention output.
  - Embedding gather = fp32 one-hot matmul (exact); positional add via
    fp32 identity matmul into the same PSUM accumulation.
"""

import os
import numpy as np
import ml_dtypes

import concourse.bass as bass
import concourse.mybir as mybir
import concourse.tile as tile
from concourse.bass_utils import run_bass_kernel_spmd
from contextlib import ExitStack

F32 = mybir.dt.float32
BF16 = mybir.dt.bfloat16
AF = mybir.ActivationFunctionType
OP = mybir.AluOpType

B, T, V, D, H, L = 64, 256, 65, 384, 6, 6
HD = D // H          # 64
FF = 4 * D           # 1536
EPS = 1e-5
SCALE = D ** -0.5

NCORES = 8
BPC = B // NCORES    # 8 batch elements per core
NTOK = BPC * T       # 2048 tokens per core
TT = NTOK // 128     # 16 token tiles
KT = D // 128        # 3 feature tiles
FT = FF // 128       # 12 ff tiles
NCH = 4              # 512-token chunks for feature-major matmuls
FFCH = 2             # ff processed in 2 chunks of 1024 tokens
FTOK = NTOK // FFCH  # 1024


def _split_multi_waits(nc):
    """This walrus build rejects >1 sync wait per instruction; hoist extras
    onto standalone EventSemaphore instructions on the same engine queue."""
    ctr = 0
    for func in nc.m.functions:
        for bb in func.blocks:
            insts = list(bb.instructions)
            out = []
            changed = False
            for inst in insts:
                si = inst.sync_info
                if si is not None and len(si.on_wait) > 1:
                    waits = list(si.on_wait)
                    for w in waits[:-1]:
                        ev = mybir.InstEventSemaphore(
                            name=f"splitwait_{ctr}", ins=[], outs=[])
                        ctr += 1
                        ev.engine = inst.engine
                        ev.sync_info = mybir.SyncInfo(on_wait=[w], on_update=[])
                        nc.register_instruction(ev, overwrite=True)
                        out.append(ev)
                    inst.sync_info = mybir.SyncInfo(
                        on_wait=[waits[-1]], on_update=list(si.on_update))
                    changed = True
                out.append(inst)
            if changed:
                bb.instructions = out


DBG = None  # e.g. ("x0", 16, 128, 384, "f32") stage tag set by tests
ATT_TP = os.environ.get("ATT_TP", "1") == "1"  # partition-packed l/o via tile_position


def build(n_layers=L):
    nc = bass.Bass(trn_type="TRN2", num_devices=NCORES)

    def din(name, shape, dt):
        return nc.dram_tensor(name, shape, dt, kind="ExternalInput").ap()

    onehotT = din("onehotT", [V, NTOK], F32)
    tokemb = din("tokemb", [V, D], F32)
    pos = din("pos", [T, D], F32)
    ident = din("ident", [128, 128], F32)
    trimask2 = din("trimask2", [128, 256], BF16)
    ones_row = din("ones_row", [1, 128], BF16)
    if n_layers:
        wqkv = din("wqkv", [n_layers, 128, KT, 3 * D], BF16)
        bqk = din("bqk", [n_layers, 128, 6], F32)
        wproj = din("wproj", [n_layers, 128, KT, D], BF16)
        bproj = din("bproj", [n_layers, 1, D], BF16)
        wff1 = din("wff1", [n_layers, 128, KT, FF], BF16)
        bff1 = din("bff1", [n_layers, 128, FT], F32)
        wff2 = din("wff2", [n_layers, 128, FT, D], BF16)
        bff2 = din("bff2", [n_layers, 1, D], BF16)
    whead = din("whead", [128, KT, V], BF16)
    bhead = din("bhead", [1, V], BF16)
    logits = nc.dram_tensor("logits", [NTOK, V], F32, kind="ExternalOutput").ap()
    dbg_spec = {
        "x0": (16, 128, D, F32), "xnT": (KT, 128, NTOK, BF16),
        "qT": (KT, 128, NTOK, BF16), "kT": (KT, 128, NTOK, BF16),
        "vt": (16, 128, D, BF16), "oT": (KT, 128, NTOK, BF16),
        "x1": (16, 128, D, F32), "x2": (16, 128, D, F32),
        "xnT2": (KT, 128, NTOK, BF16),
        "attE": (4, 128, 256, BF16), "attO": (4, 128, 256, F32),
    }
    dbg_ap = None
    if DBG is not None:
        n, p, c, dt = dbg_spec[DBG]
        dbg_ap = nc.dram_tensor("dbg", [n, p, c], dt, kind="ExternalOutput").ap()

    def dump(tag, tiles):
        if DBG == tag:
            for i, tl in enumerate(tiles):
                nc.sync.dma_start(out=dbg_ap[i], in_=tl)

    def dump3(tag, t3):
        if DBG == tag:
            for i in range(KT):
                nc.sync.dma_start(out=dbg_ap[i], in_=t3[:, i, :])

    with tile.TileContext(nc) as tc, ExitStack() as ctx:
        pool = lambda name, bufs: ctx.enter_context(tc.tile_pool(name=name, bufs=bufs))
        const_p = pool("const", 1)
        xres_p = pool("xres", 1)
        xnT_p = pool("xnT", 1)
        qkT_p = pool("qkT", 1)
        v_p = pool("vtok", 1)
        oT_p = pool("oT", 1)
        h_p = pool("hff", 1)
        w_p = pool("wts", 1)
        ln_p = pool("ln", 4)
        at_p = pool("attn", 4)
        cp_p = pool("cpy", 3)

        # ---- constants ----
        tri2_s = const_p.tile([128, 256], BF16)
        nc.sync.dma_start(out=tri2_s, in_=trimask2)
        ones_s = const_p.tile([1, 128], BF16)
        nc.sync.dma_start(out=ones_s, in_=ones_row)
        whead_s = const_p.tile([128, KT, V], BF16)
        nc.sync.dma_start(out=whead_s, in_=whead)
        bhead_s = const_p.tile([1, V], BF16)
        nc.sync.dma_start(out=bhead_s, in_=bhead)
        eps_t = const_p.tile([128, 1], F32)
        nc.vector.memset(eps_t, EPS)
        ones128 = const_p.tile([128, 128], BF16)
        nc.vector.memset(ones128, 1.0)

        # ---- persistent activation tiles ----
        x = [xres_p.tile([128, D], F32, tag=f"x{t}", name=f"x{t}") for t in range(TT)]
        xnT = xnT_p.tile([128, KT, NTOK], BF16, tag="xnT", name="xnT")
        qT = [qkT_p.tile([128, NTOK], BF16, tag=f"qT{k}", name=f"qT{k}") for k in range(KT)]
        kT = [qkT_p.tile([128, NTOK], BF16, tag=f"kT{k}", name=f"kT{k}") for k in range(KT)]
        vt = [v_p.tile([128, D], BF16, tag=f"v{t}", name=f"v{t}") for t in range(TT)]
        oT = [oT_p.tile([128, NTOK], BF16, tag=f"oT{k}", name=f"oT{k}") for k in range(KT)]
        hh = [h_p.tile([128, FTOK], BF16, tag=f"h{f}", name=f"h{f}") for f in range(FT)]

        # ---- embedding: x = onehot @ tok_emb + pos ----
        with tc.tile_pool(name="emb_ps", bufs=3, space="PSUM") as emb_ps, \
             tc.tile_pool(name="emb_sb", bufs=1) as emb_sb:
            oh_s = emb_sb.tile([V, NTOK], F32)
            nc.sync.dma_start(out=oh_s, in_=onehotT)
            te_s = emb_sb.tile([V, D], F32)
            nc.sync.dma_start(out=te_s, in_=tokemb)
            pos_s = emb_sb.tile([128, 2, D], F32)
            nc.sync.dma_start(out=pos_s, in_=pos.rearrange("(a p) n -> p a n", p=128))
            id_s = emb_sb.tile([128, 128], F32)
            nc.sync.dma_start(out=id_s, in_=ident)
            for t in range(TT):
                ps = emb_ps.tile([128, D], F32, tag="emb")
                nc.tensor.matmul(ps, lhsT=oh_s[:, t * 128:(t + 1) * 128],
                                 rhs=te_s, start=True, stop=False)
                nc.tensor.matmul(ps, lhsT=id_s, rhs=pos_s[:, t % 2, :],
                                 start=False, stop=True)
                nc.scalar.copy(out=x[t], in_=ps)
        dump("x0", x)

        def layernorm_and_transpose(dst_T):
            """token-major LN over x -> bf16 -> DMA-transpose into dst_T."""
            ctx2 = nc.named_scope("ln")
            ctx2.__enter__()
            for t in range(TT):
                stats = ln_p.tile([128, 6], F32, tag="stats")
                nc.vector.bn_stats(out=stats, in_=x[t])
                mv = ln_p.tile([128, 2], F32, tag="mv")
                nc.vector.bn_aggr(out=mv, in_=stats)
                rstd = ln_p.tile([128, 1], F32, tag="rstd")
                nc.scalar.activation(out=rstd, in_=mv[:, 1:2], func=AF.Sqrt,
                                     bias=eps_t)
                nc.vector.reciprocal(out=rstd, in_=rstd)
                xn16 = ln_p.tile([128, D], BF16, tag="xn16")
                nc.vector.tensor_scalar(out=xn16, in0=x[t], scalar1=mv[:, 0:1],
                                        scalar2=rstd, op0=OP.subtract, op1=OP.mult)
                nc.sync.dma_start(out=dst_T[:, :, t * 128:(t + 1) * 128],
                                  in_=xn16, transpose=True)
            ctx2.__exit__(None, None, None)

        for l in range(n_layers):
            # ---- layer weights ----
            wqkv_s = w_p.tile([128, KT, 3 * D], BF16, tag="wqkv")
            nc.gpsimd.dma_start(out=wqkv_s, in_=wqkv[l])
            bqk_s = w_p.tile([128, 6], F32, tag="bqk")
            nc.gpsimd.dma_start(out=bqk_s, in_=bqk[l])
            wproj_s = w_p.tile([128, KT, D], BF16, tag="wproj")
            nc.gpsimd.dma_start(out=wproj_s, in_=wproj[l])
            bproj_s = w_p.tile([1, D], BF16, tag="bproj")
            nc.gpsimd.dma_start(out=bproj_s, in_=bproj[l])
            wff1_s = w_p.tile([128, KT, FF], BF16, tag="wff1")
            nc.gpsimd.dma_start(out=wff1_s, in_=wff1[l])
            bff1_s = w_p.tile([128, FT], F32, tag="bff1")
            nc.gpsimd.dma_start(out=bff1_s, in_=bff1[l])
            wff2_s = w_p.tile([128, FT, D], BF16, tag="wff2")
            nc.gpsimd.dma_start(out=wff2_s, in_=wff2[l])
            bff2_s = w_p.tile([1, D], BF16, tag="bff2")
            nc.gpsimd.dma_start(out=bff2_s, in_=bff2[l])

            # ---- LN1 -> xnT ----
            layernorm_and_transpose(xnT)
            if l == 0:
                dump3("xnT", xnT)

            # ---- qT, kT feature-major ----
            with tc.tile_pool(name="qk_ps", bufs=3, space="PSUM") as qk_ps, \
                 nc.named_scope("qkv"):
                for m in range(6):  # 6 chunks of 128 over q|k (768 cols)
                    dst = qT[m] if m < KT else kT[m - KT]
                    for n in range(NCH):
                        ns = slice(n * 512, (n + 1) * 512)
                        ps = qk_ps.tile([128, 512], F32, tag="qk")
                        for k in range(KT):
                            nc.tensor.matmul(
                                ps, lhsT=wqkv_s[:, k, m * 128:(m + 1) * 128],
                                rhs=xnT[:, k, ns], start=(k == 0), stop=(k == KT - 1))
                        nc.scalar.activation(out=dst[:, ns], in_=ps, func=AF.Identity,
                                             bias=bqk_s[:, m:m + 1])

                # ---- v token-major (same psum pool scope) ----
                # v bias is folded into proj bias host-side (o/l + bv).
                for t in range(TT):
                    ps = qk_ps.tile([128, D], F32, tag="vps")
                    for k in range(KT):
                        nc.tensor.matmul(ps, lhsT=xnT[:, k, t * 128:(t + 1) * 128],
                                         rhs=wqkv_s[:, k, 2 * D:3 * D],
                                         start=(k == 0), stop=(k == KT - 1))
                    nc.vector.tensor_copy(out=vt[t], in_=ps)
            if l == 0:
                dump("qT", qT)
                dump("kT", kT)
                dump("vt", vt)

            # ---- attention: head-pair batched, partition-packed l/o ----
            # Pair p covers heads (2p, 2p+1) = rows 0:64 / 64:128 of qT[p]/kT[p].
            # sc0 cols = [h0q0 | h0q1 | h1q0 | h1q1]; sc1 cols = [h0q1k1 | h1q1k1]
            # l_ps/o_ps pack h0 in partitions 0:64, h1 in 64:128 (cols = [q0|q1]).
            with tc.tile_pool(name="sc_ps", bufs=2, space="PSUM") as sc_psp, \
                 tc.tile_pool(name="lo_ps", bufs=2, space="PSUM") as lo_psp, \
                 nc.named_scope("attn"):
                for b in range(BPC):
                    n0 = b * T
                    for p in range(H // 2):
                        # per-head score tiles in SEPARATE psum banks: the two
                        # heads' score MMs run on different PE row groups
                        # concurrently and must not drain into one bank.
                        # layout [q0|q1 vs k0 (256) | q1 vs k1 (128)]
                        scs = [sc_psp.tile([128, 384], F32, tag="scA", name="scA"),
                               sc_psp.tile([128, 384], F32, tag="scB", name="scB")]
                        ehs = [at_p.tile([128, 384], BF16, tag="eA", name="eA"),
                               at_p.tile([128, 384], BF16, tag="eB", name="eB")]
                        for hh_ in range(2):
                            r = hh_ * 64
                            nc.tensor.matmul(
                                scs[hh_][:, 0:256],
                                lhsT=kT[p][r:r + 64, n0:n0 + 128],
                                rhs=qT[p][r:r + 64, n0:n0 + 256],
                                start=True, stop=True)
                            nc.tensor.matmul(
                                scs[hh_][:, 256:384],
                                lhsT=kT[p][r:r + 64, n0 + 128:n0 + 256],
                                rhs=qT[p][r:r + 64, n0 + 128:n0 + 256],
                                start=True, stop=True)
                            nc.scalar.activation(out=ehs[hh_], in_=scs[hh_],
                                                 func=AF.Exp, scale=SCALE)
                            # causal mask on the diag blocks (q0k0, q1k1)
                            nc.vector.tensor_tensor(
                                out=ehs[hh_][:, 0:128], in0=ehs[hh_][:, 0:128],
                                in1=tri2_s[:, 0:128], op=OP.mult)
                            nc.vector.tensor_tensor(
                                out=ehs[hh_][:, 256:384],
                                in0=ehs[hh_][:, 256:384],
                                in1=tri2_s[:, 0:128], op=OP.mult)
                        if DBG == "attE" and l == 0 and b == 7 and p == 1:
                            nc.sync.dma_start(out=dbg_ap[0], in_=ehs[0][:, 0:256])
                            nc.sync.dma_start(out=dbg_ap[1], in_=ehs[1][:, 0:256])
                            nc.sync.dma_start(out=dbg_ap[2][:, 0:128],
                                              in_=ehs[0][:, 256:384])

                        l_ps = lo_psp.tile([128, 256], F32, tag="lps")
                        o_ps = lo_psp.tile([128, 256], F32, tag="ops")
                        for hh_ in range(2):
                            r = hh_ * 64
                            tp = None if hh_ == 0 else (0, 64)
                            nc.tensor.matmul(
                                l_ps[r:r + 64, 0:256], lhsT=ones128[:, 0:64],
                                rhs=ehs[hh_][:, 0:256],
                                start=True, stop=False, tile_position=tp)
                            nc.tensor.matmul(
                                l_ps[r:r + 64, 128:256], lhsT=ones128[:, 0:64],
                                rhs=ehs[hh_][:, 256:384],
                                start=False, stop=True, tile_position=tp)
                            h = 2 * p + hh_
                            nc.tensor.matmul(
                                o_ps[r:r + 64, 0:256],
                                lhsT=vt[2 * b][:, h * 64:(h + 1) * 64],
                                rhs=ehs[hh_][:, 0:256],
                                start=True, stop=False, tile_position=tp)
                            nc.tensor.matmul(
                                o_ps[r:r + 64, 128:256],
                                lhsT=vt[2 * b + 1][:, h * 64:(h + 1) * 64],
                                rhs=ehs[hh_][:, 256:384],
                                start=False, stop=True, tile_position=tp)
                        linv = at_p.tile([128, 256], F32, tag="linv")
                        nc.vector.reciprocal(out=linv, in_=l_ps)
                        nc.vector.tensor_tensor(out=oT[p][:, n0:n0 + 256],
                                                in0=o_ps, in1=linv, op=OP.mult)
                if l == 0:
                    dump("oT", oT)

            # ---- proj token-major + residual ----
            with tc.tile_pool(name="tok_ps", bufs=3, space="PSUM") as tok_ps, \
                 nc.named_scope("projff"):
                for t in range(TT):
                    ps = tok_ps.tile([128, D], F32, tag="tok")
                    for k in range(KT):
                        nc.tensor.matmul(ps, lhsT=oT[k][:, t * 128:(t + 1) * 128],
                                         rhs=wproj_s[:, k, :],
                                         start=(k == 0), stop=False)
                    nc.tensor.matmul(ps, lhsT=ones_s,
                                     rhs=bproj_s, start=False, stop=True)
                    nc.vector.tensor_tensor(out=x[t], in0=x[t], in1=ps, op=OP.add)
                if l == 0:
                    dump("x1", x)

                # ---- LN2 -> xnT ----
                layernorm_and_transpose(xnT)
                if l == 0:
                    dump3("xnT2", xnT)

                # ---- FF in two 1024-token chunks ----
                for ch in range(FFCH):
                    c0 = ch * FTOK
                    for f in range(FT):
                        for n in range(FTOK // 512):
                            ns = slice(c0 + n * 512, c0 + (n + 1) * 512)
                            hs = slice(n * 512, (n + 1) * 512)
                            ps = tok_ps.tile([128, 512], F32, tag="ff1")
                            for k in range(KT):
                                nc.tensor.matmul(
                                    ps, lhsT=wff1_s[:, k, f * 128:(f + 1) * 128],
                                    rhs=xnT[:, k, ns], start=(k == 0),
                                    stop=(k == KT - 1))
                            nc.scalar.activation(out=hh[f][:, hs], in_=ps,
                                                 func=AF.Relu,
                                                 bias=bff1_s[:, f:f + 1])
                    for tt in range(FTOK // 128):
                        t = ch * (FTOK // 128) + tt
                        ps = tok_ps.tile([128, D], F32, tag="tok")
                        for f in range(FT):
                            nc.tensor.matmul(
                                ps, lhsT=hh[f][:, tt * 128:(tt + 1) * 128],
                                rhs=wff2_s[:, f, :], start=(f == 0), stop=False)
                        nc.tensor.matmul(ps, lhsT=ones_s,
                                         rhs=bff2_s, start=False, stop=True)
                        nc.vector.tensor_tensor(out=x[t], in0=x[t], in1=ps, op=OP.add)
            if l == 0:
                dump("x2", x)

        # ---- final LN + head ----
        layernorm_and_transpose(xnT)
        with tc.tile_pool(name="hd_ps", bufs=3, space="PSUM") as hd_ps:
            for t in range(TT):
                ps = hd_ps.tile([128, V], F32, tag="hd")
                for k in range(KT):
                    nc.tensor.matmul(ps, lhsT=xnT[:, k, t * 128:(t + 1) * 128],
                                     rhs=whead_s[:, k, :], start=(k == 0), stop=False)
                nc.tensor.matmul(ps, lhsT=ones_s,
                                 rhs=bhead_s, start=False, stop=True)
                lt = cp_p.tile([128, V], F32, tag="logit")
                nc.scalar.copy(out=lt, in_=ps)
                nc.sync.dma_start(out=logits[t * 128:(t + 1) * 128, :], in_=lt)

    _split_multi_waits(nc)
    return nc


def prepare_host_inputs(idx, tok_emb, pos_emb, ln1_w, ln1_b, wq, wk, wv,
                        proj_w, proj_b, ln2_w, ln2_b, ff_w1, ff_b1, ff_w2,
                        ff_b2, lnf_w, lnf_b, head_w, head_b, n_layers=L):
    f32 = np.float32
    bf = ml_dtypes.bfloat16

    def kt_tiles(w, ncols):  # [D, ncols] -> [128, KT, ncols]
        return np.ascontiguousarray(
            w.reshape(KT, 128, ncols).transpose(1, 0, 2))

    wqkv_l, bqk_l = [], []
    wproj_l, bproj_l = [], []
    wff1_l, bff1_l, wff2_l, bff2_l = [], [], [], []
    for l in range(n_layers):
        # [H, D, HD] -> [D, H*HD]
        q2 = wq[l].transpose(1, 0, 2).reshape(D, D).astype(f32)
        k2 = wk[l].transpose(1, 0, 2).reshape(D, D).astype(f32)
        v2 = wv[l].transpose(1, 0, 2).reshape(D, D).astype(f32)
        qf = ln1_w[l][:, None] * q2
        kf = ln1_w[l][:, None] * k2
        vf = ln1_w[l][:, None] * v2
        bq = ln1_b[l] @ q2
        bk = ln1_b[l] @ k2
        bvv = ln1_b[l] @ v2
        wqkv_l.append(kt_tiles(np.concatenate([qf, kf, vf], axis=1), 3 * D))
        # col m (m<3) = bq[m*128+p]; col 3+m = bk[m*128+p]
        bqk_l.append(np.concatenate(
            [bq.reshape(KT, 128).T, bk.reshape(KT, 128).T], axis=1))
        wproj_l.append(kt_tiles(proj_w[l].astype(f32), D))
        # v bias folded through attention (o/l + bv) into the proj bias.
        bproj_l.append((proj_b[l] + bvv @ proj_w[l]).reshape(1, D))
        w1f = ln2_w[l][:, None] * ff_w1[l].astype(f32)
        b1f = ff_b1[l] + ln2_b[l] @ ff_w1[l]
        wff1_l.append(kt_tiles(w1f, FF))
        bff1_l.append(np.ascontiguousarray(
            b1f.reshape(FT, 128).T).astype(f32))
        wff2_l.append(np.ascontiguousarray(
            ff_w2[l].reshape(FT, 128, D).transpose(1, 0, 2)).astype(f32))
        bff2_l.append(ff_b2[l].reshape(1, D))

    whf = lnf_w[:, None] * head_w.astype(f32)
    bhf = head_b + lnf_b @ head_w

    def stk(lst, shape, dt):
        if lst:
            return np.stack(lst).astype(dt)
        return np.zeros((0,) + shape, dt)

    tri = np.triu(np.ones((128, 128), f32))  # [s,t] valid s<=t
    shared = {
        "tokemb": np.asarray(tok_emb, f32),
        "pos": np.asarray(pos_emb, f32),
        "ident": np.eye(128, dtype=f32),
        "trimask2": np.concatenate([tri, tri], axis=1).astype(bf),
        "ones_row": np.ones((1, 128), bf),
        "wqkv": stk(wqkv_l, (128, KT, 3 * D), bf),
        "bqk": stk(bqk_l, (128, 6), f32),
        "wproj": stk(wproj_l, (128, KT, D), bf),
        "bproj": stk(bproj_l, (1, D), bf),
        "wff1": stk(wff1_l, (128, KT, FF), bf),
        "bff1": stk(bff1_l, (128, FT), f32),
        "wff2": stk(wff2_l, (128, FT, D), bf),
        "bff2": stk(bff2_l, (1, D), bf),
        "whead": kt_tiles(whf, V).astype(bf),
        "bhead": bhf.reshape(1, V).astype(bf),
    }
    idx = np.asarray(idx)
    in_maps = []
    for c in range(NCORES):
        oh = np.zeros((V, NTOK), f32)
        sl = idx[c * BPC:(c + 1) * BPC].reshape(NTOK)  # token n = b*T + t
        oh[sl, np.arange(NTOK)] = 1.0
        in_maps.append({**shared, "onehotT": oh})
    return in_maps


_CACHED = {}


def kernel(**inputs):
    n_layers = L
    if "nc" not in _CACHED:
        _CACHED["nc"] = build(n_layers)
    nc = _CACHED["nc"]
    in_maps = prepare_host_inputs(**inputs, n_layers=n_layers)
    res = run_bass_kernel_spmd(nc, in_maps, list(range(NCORES)))
    out = np.concatenate(
        [res.results[c]["logits"].reshape(BPC, T, V) for c in range(NCORES)],
        axis=0)
    return out



# revision 18
# speedup vs baseline: 1.3000x; 1.0593x over previous
"""Trainium2 Bass kernel for a 6-layer GPT-style transformer
(B=64, T=256, V=65, D=384, H=6, FF=1536), data-parallel over batch on 8
NeuronCores (8 batch elements = 2048 tokens per core).

Design:
  - Residual stream x lives token-major in SBUF as 16 fp32 tiles [128, 384].
  - LayerNorm in token-major via bn_stats/bn_aggr + fused (x-m)*rstd, with
    ln scale/bias folded into the following matmul weights host-side.
    Output cast to bf16 and DMA-transposed (xbar) to feature-major xnT.
  - Big matmuls run in bf16 (fp32 PSUM accumulate):
      * weights-stationary  -> feature-major outputs (qT, kT, ff1 hidden)
      * activation-stationary -> token-major outputs (v, proj, ff2, head)
  - Biases are applied either as per-partition ACT Identity/Relu bias
    (feature-major) or as K=1 ones-row matmuls accumulated in PSUM
    (token-major).
  - Attention per (batch, head) entirely feature-major with causal
    quadrant skipping; softmax normalization folded into the PSUM->SBUF
    copy of the attention output.
  - Embedding gather = fp32 one-hot matmul (exact); positional add via
    fp32 identity matmul into the same PSUM accumulation.
"""

import os
import numpy as np
import ml_dtypes

import concourse.bass as bass
import concourse.mybir as mybir
import concourse.tile as tile
from concourse.bass_utils import run_bass_kernel_spmd
from contextlib import ExitStack

F32 = mybir.dt.float32
BF16 = mybir.dt.bfloat16
AF = mybir.ActivationFunctionType
OP = mybir.AluOpType

B, T, V, D, H, L = 64, 256, 65, 384, 6, 6
HD = D // H          # 64
FF = 4 * D           # 1536
EPS = 1e-5
SCALE = D ** -0.5

NCORES = 8
BPC = B // NCORES    # 8 batch elements per core
NTOK = BPC * T       # 2048 tokens per core
TT = NTOK // 128     # 16 token tiles
KT = D // 128        # 3 feature tiles
FT = FF // 128       # 12 ff tiles
NCH = 4              # 512-token chunks for feature-major matmuls
FFCH = 2             # ff processed in 2 chunks of 1024 tokens
FTOK = NTOK // FFCH  # 1024


def _split_multi_waits(nc):
    """This walrus build rejects >1 sync wait per instruction; hoist extras
    onto standalone EventSemaphore instructions on the same engine queue."""
    ctr = 0
    for func in nc.m.functions:
        for bb in func.blocks:
            insts = list(bb.instructions)
            out = []
            changed = False
            for inst in insts:
                si = inst.sync_info
                if si is not None and len(si.on_wait) > 1:
                    waits = list(si.on_wait)
                    for w in waits[:-1]:
                        ev = mybir.InstEventSemaphore(
                            name=f"splitwait_{ctr}", ins=[], outs=[])
                        ctr += 1
                        ev.engine = inst.engine
                        ev.sync_info = mybir.SyncInfo(on_wait=[w], on_update=[])
                        nc.register_instruction(ev, overwrite=True)
                        out.append(ev)
                    inst.sync_info = mybir.SyncInfo(
                        on_wait=[waits[-1]], on_update=list(si.on_update))
                    changed = True
                out.append(inst)
            if changed:
                bb.instructions = out


DBG = None  # e.g. ("x0", 16, 128, 384, "f32") stage tag set by tests
ATT_TP = os.environ.get("ATT_TP", "1") == "1"  # partition-packed l/o via tile_position
RECIPFAST = os.environ.get("RECIPFAST", "1") == "1"


def build(n_layers=L):
    nc = bass.Bass(trn_type="TRN2", num_devices=NCORES)

    def din(name, shape, dt):
        return nc.dram_tensor(name, shape, dt, kind="ExternalInput").ap()

    onehotT = din("onehotT", [V, NTOK], F32)
    tokemb = din("tokemb", [V, D], F32)
    pos = din("pos", [T, D], F32)
    ident = din("ident", [128, 128], F32)
    trimask2 = din("trimask2", [128, 256], BF16)
    ones_row = din("ones_row", [1, 128], BF16)
    if n_layers:
        wqkv = din("wqkv", [n_layers, 128, KT, 3 * D], BF16)
        bqk = din("bqk", [n_layers, 128, 6], F32)
        wproj = din("wproj", [n_layers, 128, KT, D], BF16)
        bproj = din("bproj", [n_layers, 1, D], BF16)
        wff1 = din("wff1", [n_layers, 128, KT, FF], BF16)
        bff1 = din("bff1", [n_layers, 128, FT], F32)
        wff2 = din("wff2", [n_layers, 128, FT, D], BF16)
        bff2 = din("bff2", [n_layers, 1, D], BF16)
    whead = din("whead", [128, KT, V], BF16)
    bhead = din("bhead", [1, V], BF16)
    logits = nc.dram_tensor("logits", [NTOK, V], F32, kind="ExternalOutput").ap()
    dbg_spec = {
        "x0": (16, 128, D, F32), "xnT": (KT, 128, NTOK, BF16),
        "qT": (KT, 128, NTOK, BF16), "kT": (KT, 128, NTOK, BF16),
        "vt": (16, 128, D, BF16), "oT": (KT, 128, NTOK, BF16),
        "x1": (16, 128, D, F32), "x2": (16, 128, D, F32),
        "xnT2": (KT, 128, NTOK, BF16),
        "attE": (4, 128, 256, BF16), "attO": (4, 128, 256, F32),
    }
    dbg_ap = None
    if DBG is not None:
        n, p, c, dt = dbg_spec[DBG]
        dbg_ap = nc.dram_tensor("dbg", [n, p, c], dt, kind="ExternalOutput").ap()

    def dump(tag, tiles):
        if DBG == tag:
            for i, tl in enumerate(tiles):
                nc.sync.dma_start(out=dbg_ap[i], in_=tl)

    def dump3(tag, t3):
        if DBG == tag:
            for i in range(KT):
                nc.sync.dma_start(out=dbg_ap[i], in_=t3[:, i, :])

    with tile.TileContext(nc) as tc, ExitStack() as ctx:
        pool = lambda name, bufs: ctx.enter_context(tc.tile_pool(name=name, bufs=bufs))
        const_p = pool("const", 1)
        xres_p = pool("xres", 1)
        xnT_p = pool("xnT", 1)
        qkT_p = pool("qkT", 1)
        v_p = pool("vtok", 1)
        oT_p = pool("oT", 1)
        h_p = pool("hff", 1)
        w_p = pool("wts", 1)
        ln_p = pool("ln", 4)
        at_p = pool("attn", 4)
        cp_p = pool("cpy", 3)

        # ---- constants ----
        tri2_s = const_p.tile([128, 256], BF16)
        nc.sync.dma_start(out=tri2_s, in_=trimask2)
        ones_s = const_p.tile([1, 128], BF16)
        nc.sync.dma_start(out=ones_s, in_=ones_row)
        whead_s = const_p.tile([128, KT, V], BF16)
        nc.sync.dma_start(out=whead_s, in_=whead)
        bhead_s = const_p.tile([1, V], BF16)
        nc.sync.dma_start(out=bhead_s, in_=bhead)
        eps_t = const_p.tile([128, 1], F32)
        nc.vector.memset(eps_t, EPS)
        ones128 = const_p.tile([128, 128], BF16)
        nc.vector.memset(ones128, 1.0)

        # ---- persistent activation tiles ----
        x = [xres_p.tile([128, D], F32, tag=f"x{t}", name=f"x{t}") for t in range(TT)]
        xnT = xnT_p.tile([128, KT, NTOK], BF16, tag="xnT", name="xnT")
        qT = [qkT_p.tile([128, NTOK], BF16, tag=f"qT{k}", name=f"qT{k}") for k in range(KT)]
        kT = [qkT_p.tile([128, NTOK], BF16, tag=f"kT{k}", name=f"kT{k}") for k in range(KT)]
        vt = [v_p.tile([128, D], BF16, tag=f"v{t}", name=f"v{t}") for t in range(TT)]
        oT = [oT_p.tile([128, NTOK], BF16, tag=f"oT{k}", name=f"oT{k}") for k in range(KT)]
        hh = [h_p.tile([128, FTOK], BF16, tag=f"h{f}", name=f"h{f}") for f in range(FT)]

        # ---- embedding: x = onehot @ tok_emb + pos ----
        with tc.tile_pool(name="emb_ps", bufs=3, space="PSUM") as emb_ps, \
             tc.tile_pool(name="emb_sb", bufs=1) as emb_sb:
            oh_s = emb_sb.tile([V, NTOK], F32)
            nc.sync.dma_start(out=oh_s, in_=onehotT)
            te_s = emb_sb.tile([V, D], F32)
            nc.sync.dma_start(out=te_s, in_=tokemb)
            pos_s = emb_sb.tile([128, 2, D], F32)
            nc.sync.dma_start(out=pos_s, in_=pos.rearrange("(a p) n -> p a n", p=128))
            id_s = emb_sb.tile([128, 128], F32)
            nc.sync.dma_start(out=id_s, in_=ident)
            for t in range(TT):
                ps = emb_ps.tile([128, D], F32, tag="emb")
                nc.tensor.matmul(ps, lhsT=oh_s[:, t * 128:(t + 1) * 128],
                                 rhs=te_s, start=True, stop=False)
                nc.tensor.matmul(ps, lhsT=id_s, rhs=pos_s[:, t % 2, :],
                                 start=False, stop=True)
                nc.scalar.copy(out=x[t], in_=ps)
        dump("x0", x)

        def layernorm_and_transpose(dst_T):
            """token-major LN over x -> bf16 -> DMA-transpose into dst_T."""
            ctx2 = nc.named_scope("ln")
            ctx2.__enter__()
            for t in range(TT):
                stats = ln_p.tile([128, 6], F32, tag="stats")
                nc.vector.bn_stats(out=stats, in_=x[t])
                mv = ln_p.tile([128, 2], F32, tag="mv")
                nc.vector.bn_aggr(out=mv, in_=stats)
                rstd = ln_p.tile([128, 1], F32, tag="rstd")
                nc.scalar.activation(out=rstd, in_=mv[:, 1:2], func=AF.Sqrt,
                                     bias=eps_t)
                nc.vector.reciprocal(out=rstd, in_=rstd)
                xn16 = ln_p.tile([128, D], BF16, tag="xn16")
                nc.vector.tensor_scalar(out=xn16, in0=x[t], scalar1=mv[:, 0:1],
                                        scalar2=rstd, op0=OP.subtract, op1=OP.mult)
                nc.sync.dma_start(out=dst_T[:, :, t * 128:(t + 1) * 128],
                                  in_=xn16, transpose=True)
            ctx2.__exit__(None, None, None)

        for l in range(n_layers):
            # ---- layer weights ----
            wqkv_s = w_p.tile([128, KT, 3 * D], BF16, tag="wqkv")
            nc.gpsimd.dma_start(out=wqkv_s, in_=wqkv[l])
            bqk_s = w_p.tile([128, 6], F32, tag="bqk")
            nc.gpsimd.dma_start(out=bqk_s, in_=bqk[l])
            wproj_s = w_p.tile([128, KT, D], BF16, tag="wproj")
            nc.gpsimd.dma_start(out=wproj_s, in_=wproj[l])
            bproj_s = w_p.tile([1, D], BF16, tag="bproj")
            nc.gpsimd.dma_start(out=bproj_s, in_=bproj[l])
            wff1_s = w_p.tile([128, KT, FF], BF16, tag="wff1")
            nc.gpsimd.dma_start(out=wff1_s, in_=wff1[l])
            bff1_s = w_p.tile([128, FT], F32, tag="bff1")
            nc.gpsimd.dma_start(out=bff1_s, in_=bff1[l])
            wff2_s = w_p.tile([128, FT, D], BF16, tag="wff2")
            nc.gpsimd.dma_start(out=wff2_s, in_=wff2[l])
            bff2_s = w_p.tile([1, D], BF16, tag="bff2")
            nc.gpsimd.dma_start(out=bff2_s, in_=bff2[l])

            # ---- LN1 -> xnT ----
            layernorm_and_transpose(xnT)
            if l == 0:
                dump3("xnT", xnT)

            # ---- qT, kT feature-major ----
            with tc.tile_pool(name="qk_ps", bufs=3, space="PSUM") as qk_ps, \
                 nc.named_scope("qkv"):
                for m in range(6):  # 6 chunks of 128 over q|k (768 cols)
                    dst = qT[m] if m < KT else kT[m - KT]
                    for n in range(NCH):
                        ns = slice(n * 512, (n + 1) * 512)
                        ps = qk_ps.tile([128, 512], F32, tag="qk")
                        for k in range(KT):
                            nc.tensor.matmul(
                                ps, lhsT=wqkv_s[:, k, m * 128:(m + 1) * 128],
                                rhs=xnT[:, k, ns], start=(k == 0), stop=(k == KT - 1))
                        nc.scalar.activation(out=dst[:, ns], in_=ps, func=AF.Identity,
                                             bias=bqk_s[:, m:m + 1])

                # ---- v token-major (same psum pool scope) ----
                # v bias is folded into proj bias host-side (o/l + bv).
                for t in range(TT):
                    ps = qk_ps.tile([128, D], F32, tag="vps")
                    for k in range(KT):
                        nc.tensor.matmul(ps, lhsT=xnT[:, k, t * 128:(t + 1) * 128],
                                         rhs=wqkv_s[:, k, 2 * D:3 * D],
                                         start=(k == 0), stop=(k == KT - 1))
                    nc.vector.tensor_copy(out=vt[t], in_=ps)
            if l == 0:
                dump("qT", qT)
                dump("kT", kT)
                dump("vt", vt)

            # ---- attention: head-pair batched, partition-packed l/o ----
            # Pair p covers heads (2p, 2p+1) = rows 0:64 / 64:128 of qT[p]/kT[p].
            # sc0 cols = [h0q0 | h0q1 | h1q0 | h1q1]; sc1 cols = [h0q1k1 | h1q1k1]
            # l_ps/o_ps pack h0 in partitions 0:64, h1 in 64:128 (cols = [q0|q1]).
            with tc.tile_pool(name="sc_ps", bufs=2, space="PSUM") as sc_psp, \
                 tc.tile_pool(name="lo_ps", bufs=2, space="PSUM") as lo_psp, \
                 nc.named_scope("attn"):
                for b in range(BPC):
                    n0 = b * T
                    for p in range(H // 2):
                        # per-head score tiles in SEPARATE psum banks: the two
                        # heads' score MMs run on different PE row groups
                        # concurrently and must not drain into one bank.
                        # layout [q0|q1 vs k0 (256) | q1 vs k1 (128)]
                        scs = [sc_psp.tile([128, 384], F32, tag="scA", name="scA"),
                               sc_psp.tile([128, 384], F32, tag="scB", name="scB")]
                        ehs = [at_p.tile([128, 384], BF16, tag="eA", name="eA"),
                               at_p.tile([128, 384], BF16, tag="eB", name="eB")]
                        for hh_ in range(2):
                            r = hh_ * 64
                            nc.tensor.matmul(
                                scs[hh_][:, 0:256],
                                lhsT=kT[p][r:r + 64, n0:n0 + 128],
                                rhs=qT[p][r:r + 64, n0:n0 + 256],
                                start=True, stop=True)
                            nc.tensor.matmul(
                                scs[hh_][:, 256:384],
                                lhsT=kT[p][r:r + 64, n0 + 128:n0 + 256],
                                rhs=qT[p][r:r + 64, n0 + 128:n0 + 256],
                                start=True, stop=True)
                            nc.scalar.activation(out=ehs[hh_], in_=scs[hh_],
                                                 func=AF.Exp, scale=SCALE)
                            # causal mask on the diag blocks (q0k0, q1k1);
                            # gpsimd: it's otherwise idle, DVE is loaded.
                            nc.gpsimd.tensor_tensor(
                                out=ehs[hh_][:, 0:128], in0=ehs[hh_][:, 0:128],
                                in1=tri2_s[:, 0:128], op=OP.mult)
                            nc.gpsimd.tensor_tensor(
                                out=ehs[hh_][:, 256:384],
                                in0=ehs[hh_][:, 256:384],
                                in1=tri2_s[:, 0:128], op=OP.mult)
                        if DBG == "attE" and l == 0 and b == 7 and p == 1:
                            nc.sync.dma_start(out=dbg_ap[0], in_=ehs[0][:, 0:256])
                            nc.sync.dma_start(out=dbg_ap[1], in_=ehs[1][:, 0:256])
                            nc.sync.dma_start(out=dbg_ap[2][:, 0:128],
                                              in_=ehs[0][:, 256:384])

                        l_ps = lo_psp.tile([128, 256], F32, tag="lps")
                        o_ps = lo_psp.tile([128, 256], F32, tag="ops")
                        for hh_ in range(2):
                            r = hh_ * 64
                            tp = None if hh_ == 0 else (0, 64)
                            nc.tensor.matmul(
                                l_ps[r:r + 64, 0:256], lhsT=ones128[:, 0:64],
                                rhs=ehs[hh_][:, 0:256],
                                start=True, stop=False, tile_position=tp)
                            nc.tensor.matmul(
                                l_ps[r:r + 64, 128:256], lhsT=ones128[:, 0:64],
                                rhs=ehs[hh_][:, 256:384],
                                start=False, stop=True, tile_position=tp)
                            h = 2 * p + hh_
                            nc.tensor.matmul(
                                o_ps[r:r + 64, 0:256],
                                lhsT=vt[2 * b][:, h * 64:(h + 1) * 64],
                                rhs=ehs[hh_][:, 0:256],
                                start=True, stop=False, tile_position=tp)
                            nc.tensor.matmul(
                                o_ps[r:r + 64, 128:256],
                                lhsT=vt[2 * b + 1][:, h * 64:(h + 1) * 64],
                                rhs=ehs[hh_][:, 256:384],
                                start=False, stop=True, tile_position=tp)
                        # linv = 1/l via exp(-ln(l)) on ScalarE (same table set
                        # as the attention exp; DVE reciprocal is ~7 cyc/elem).
                        linv = at_p.tile([128, 256], F32, tag="linv")
                        lnl = at_p.tile([128, 256], F32, tag="lnl")
                        nc.scalar.activation(out=lnl, in_=l_ps, func=AF.Ln)
                        nc.scalar.activation(out=linv, in_=lnl, func=AF.Exp,
                                             scale=-1.0)
                        nc.vector.tensor_tensor(out=oT[p][:, n0:n0 + 256],
                                                in0=o_ps, in1=linv, op=OP.mult)
                if l == 0:
                    dump("oT", oT)

            # ---- proj token-major + residual ----
            with tc.tile_pool(name="tok_ps", bufs=3, space="PSUM") as tok_ps, \
                 nc.named_scope("projff"):
                for t in range(TT):
                    ps = tok_ps.tile([128, D], F32, tag="tok")
                    for k in range(KT):
                        nc.tensor.matmul(ps, lhsT=oT[k][:, t * 128:(t + 1) * 128],
                                         rhs=wproj_s[:, k, :],
                                         start=(k == 0), stop=False)
                    nc.tensor.matmul(ps, lhsT=ones_s,
                                     rhs=bproj_s, start=False, stop=True)
                    nc.vector.tensor_tensor(out=x[t], in0=x[t], in1=ps, op=OP.add)
                if l == 0:
                    dump("x1", x)

                # ---- LN2 -> xnT ----
                layernorm_and_transpose(xnT)
                if l == 0:
                    dump3("xnT2", xnT)

                # ---- FF in two 1024-token chunks ----
                for ch in range(FFCH):
                    c0 = ch * FTOK
                    for f in range(FT):
                        for n in range(FTOK // 512):
                            ns = slice(c0 + n * 512, c0 + (n + 1) * 512)
                            hs = slice(n * 512, (n + 1) * 512)
                            ps = tok_ps.tile([128, 512], F32, tag="ff1")
                            for k in range(KT):
                                nc.tensor.matmul(
                                    ps, lhsT=wff1_s[:, k, f * 128:(f + 1) * 128],
                                    rhs=xnT[:, k, ns], start=(k == 0),
                                    stop=(k == KT - 1))
                            nc.scalar.activation(out=hh[f][:, hs], in_=ps,
                                                 func=AF.Relu,
                                                 bias=bff1_s[:, f:f + 1])
                    for tt in range(FTOK // 128):
                        t = ch * (FTOK // 128) + tt
                        ps = tok_ps.tile([128, D], F32, tag="tok")
                        for f in range(FT):
                            nc.tensor.matmul(
                                ps, lhsT=hh[f][:, tt * 128:(tt + 1) * 128],
                                rhs=wff2_s[:, f, :], start=(f == 0), stop=False)
                        nc.tensor.matmul(ps, lhsT=ones_s,
                                         rhs=bff2_s, start=False, stop=True)
                        nc.vector.tensor_tensor(out=x[t], in0=x[t], in1=ps, op=OP.add)
            if l == 0:
                dump("x2", x)

        # ---- final LN + head ----
        layernorm_and_transpose(xnT)
        with tc.tile_pool(name="hd_ps", bufs=3, space="PSUM") as hd_ps:
            for t in range(TT):
                ps = hd_ps.tile([128, V], F32, tag="hd")
                for k in range(KT):
                    nc.tensor.matmul(ps, lhsT=xnT[:, k, t * 128:(t + 1) * 128],
                                     rhs=whead_s[:, k, :], start=(k == 0), stop=False)
                nc.tensor.matmul(ps, lhsT=ones_s,
                                 rhs=bhead_s, start=False, stop=True)
                lt = cp_p.tile([128, V], F32, tag="logit")
                nc.scalar.copy(out=lt, in_=ps)
                nc.sync.dma_start(out=logits[t * 128:(t + 1) * 128, :], in_=lt)

    _split_multi_waits(nc)
    return nc


def prepare_host_inputs(idx, tok_emb, pos_emb, ln1_w, ln1_b, wq, wk, wv,
                        proj_w, proj_b, ln2_w, ln2_b, ff_w1, ff_b1, ff_w2,
                        ff_b2, lnf_w, lnf_b, head_w, head_b, n_layers=L):
    f32 = np.float32
    bf = ml_dtypes.bfloat16

    def kt_tiles(w, ncols):  # [D, ncols] -> [128, KT, ncols]
        return np.ascontiguousarray(
            w.reshape(KT, 128, ncols).transpose(1, 0, 2))

    wqkv_l, bqk_l = [], []
    wproj_l, bproj_l = [], []
    wff1_l, bff1_l, wff2_l, bff2_l = [], [], [], []
    for l in range(n_layers):
        # [H, D, HD] -> [D, H*HD]
        q2 = wq[l].transpose(1, 0, 2).reshape(D, D).astype(f32)
        k2 = wk[l].transpose(1, 0, 2).reshape(D, D).astype(f32)
        v2 = wv[l].transpose(1, 0, 2).reshape(D, D).astype(f32)
        qf = ln1_w[l][:, None] * q2
        kf = ln1_w[l][:, None] * k2
        vf = ln1_w[l][:, None] * v2
        bq = ln1_b[l] @ q2
        bk = ln1_b[l] @ k2
        bvv = ln1_b[l] @ v2
        wqkv_l.append(kt_tiles(np.concatenate([qf, kf, vf], axis=1), 3 * D))
        # col m (m<3) = bq[m*128+p]; col 3+m = bk[m*128+p]
        bqk_l.append(np.concatenate(
            [bq.reshape(KT, 128).T, bk.reshape(KT, 128).T], axis=1))
        wproj_l.append(kt_tiles(proj_w[l].astype(f32), D))
        # v bias folded through attention (o/l + bv) into the proj bias.
        bproj_l.append((proj_b[l] + bvv @ proj_w[l]).reshape(1, D))
        w1f = ln2_w[l][:, None] * ff_w1[l].astype(f32)
        b1f = ff_b1[l] + ln2_b[l] @ ff_w1[l]
        wff1_l.append(kt_tiles(w1f, FF))
        bff1_l.append(np.ascontiguousarray(
            b1f.reshape(FT, 128).T).astype(f32))
        wff2_l.append(np.ascontiguousarray(
            ff_w2[l].reshape(FT, 128, D).transpose(1, 0, 2)).astype(f32))
        bff2_l.append(ff_b2[l].reshape(1, D))

    whf = lnf_w[:, None] * head_w.astype(f32)
    bhf = head_b + lnf_b @ head_w

    def stk(lst, shape, dt):
        if lst:
            return np.stack(lst).astype(dt)
        return np.zeros((0,) + shape, dt)

    tri = np.triu(np.ones((128, 128), f32))  # [s,t] valid s<=t
    shared = {
        "tokemb": np.asarray(tok_emb, f32),
        "pos": np.asarray(pos_emb, f32),
        "ident": np.eye(128, dtype=f32),
        "trimask2": np.concatenate([tri, tri], axis=1).astype(bf),
        "ones_row": np.ones((1, 128), bf),
        "wqkv": stk(wqkv_l, (128, KT, 3 * D), bf),
        "bqk": stk(bqk_l, (128, 6), f32),
        "wproj": stk(wproj_l, (128, KT, D), bf),
        "bproj": stk(bproj_l, (1, D), bf),
        "wff1": stk(wff1_l, (128, KT, FF), bf),
        "bff1": stk(bff1_l, (128, FT), f32),
        "wff2": stk(wff2_l, (128, FT, D), bf),
        "bff2": stk(bff2_l, (1, D), bf),
        "whead": kt_tiles(whf, V).astype(bf),
        "bhead": bhf.reshape(1, V).astype(bf),
    }
    idx = np.asarray(idx)
    in_maps = []
    for c in range(NCORES):
        oh = np.zeros((V, NTOK), f32)
        sl = idx[c * BPC:(c + 1) * BPC].reshape(NTOK)  # token n = b*T + t
        oh[sl, np.arange(NTOK)] = 1.0
        in_maps.append({**shared, "onehotT": oh})
    return in_maps


_CACHED = {}


def kernel(**inputs):
    n_layers = L
    if "nc" not in _CACHED:
        _CACHED["nc"] = build(n_layers)
    nc = _CACHED["nc"]
    in_maps = prepare_host_inputs(**inputs, n_layers=n_layers)
    res = run_bass_kernel_spmd(nc, in_maps, list(range(NCORES)))
    out = np.concatenate(
        [res.results[c]["logits"].reshape(BPC, T, V) for c in range(NCORES)],
        axis=0)
    return out



# revision 35
# speedup vs baseline: 1.5504x; 1.1926x over previous
"""Trainium2 Bass kernel for a 6-layer GPT-style transformer
(B=64, T=256, V=65, D=384, H=6, FF=1536), data-parallel over batch on 8
NeuronCores (8 batch elements = 2048 tokens per core).

Design:
  - Residual stream x lives token-major in SBUF as 16 fp32 tiles [128, 384].
  - LayerNorm in token-major via bn_stats/bn_aggr + fused (x-m)*rstd, with
    ln scale/bias folded into the following matmul weights host-side.
    Output cast to bf16 and DMA-transposed (xbar) to feature-major xnT.
  - Big matmuls run in bf16 (fp32 PSUM accumulate):
      * weights-stationary  -> feature-major outputs (qT, kT, ff1 hidden)
      * activation-stationary -> token-major outputs (v, proj, ff2, head)
  - Biases are applied either as per-partition ACT Identity/Relu bias
    (feature-major) or as K=1 ones-row matmuls accumulated in PSUM
    (token-major).
  - Attention per (batch, head) entirely feature-major with causal
    quadrant skipping; softmax normalization folded into the PSUM->SBUF
    copy of the attention output.
  - Embedding gather = fp32 one-hot matmul (exact); positional add via
    fp32 identity matmul into the same PSUM accumulation.
"""

import os
import numpy as np
import ml_dtypes

import concourse.bass as bass
import concourse.mybir as mybir
import concourse.tile as tile
from concourse.bass_utils import run_bass_kernel_spmd
from contextlib import ExitStack

F32 = mybir.dt.float32
BF16 = mybir.dt.bfloat16
FP8 = mybir.dt.float8e4
DR = mybir.MatmulPerfMode.DoubleRow
AF = mybir.ActivationFunctionType
OP = mybir.AluOpType

B, T, V, D, H, L = 64, 256, 65, 384, 6, 6
HD = D // H          # 64
FF = 4 * D           # 1536
EPS = 1e-5
SCALE = D ** -0.5
XSCALE = 256.0  # residual stream is carried as 256*x; LN is scale-invariant

NCORES = 8
BPC = B // NCORES    # 8 batch elements per core
NTOK = BPC * T       # 2048 tokens per core
TT = NTOK // 128     # 16 token tiles
KT = D // 128        # 3 feature tiles
FT = FF // 128       # 12 ff tiles
NCH = 4              # 512-token chunks for feature-major matmuls
FFCH = 2             # ff processed in 2 chunks of 1024 tokens
FTOK = NTOK // FFCH  # 1024


def _split_multi_waits(nc):
    """This walrus build rejects >1 sync wait per instruction; hoist extras
    onto standalone EventSemaphore instructions on the same engine queue."""
    ctr = 0
    for func in nc.m.functions:
        for bb in func.blocks:
            insts = list(bb.instructions)
            out = []
            changed = False
            for inst in insts:
                si = inst.sync_info
                if si is not None and len(si.on_wait) > 1:
                    waits = list(si.on_wait)
                    for w in waits[:-1]:
                        ev = mybir.InstEventSemaphore(
                            name=f"splitwait_{ctr}", ins=[], outs=[])
                        ctr += 1
                        ev.engine = inst.engine
                        ev.sync_info = mybir.SyncInfo(on_wait=[w], on_update=[])
                        nc.register_instruction(ev, overwrite=True)
                        out.append(ev)
                    inst.sync_info = mybir.SyncInfo(
                        on_wait=[waits[-1]], on_update=list(si.on_update))
                    changed = True
                out.append(inst)
            if changed:
                bb.instructions = out


DBG = None  # e.g. ("x0", 16, 128, 384, "f32") stage tag set by tests
ATT_TP = os.environ.get("ATT_TP", "1") == "1"  # partition-packed l/o via tile_position
RECIPFAST = os.environ.get("RECIPFAST", "1") == "1"


def build(n_layers=L, with_bias=True):
    nc = bass.Bass(trn_type="TRN2", num_devices=NCORES)

    def din(name, shape, dt):
        return nc.dram_tensor(name, shape, dt, kind="ExternalInput").ap()

    onehotT = din("onehotT", [V, NTOK], BF16)
    tokemb = din("tokemb", [V, D], BF16)
    pos = din("pos", [T, D], BF16)
    ident = din("ident", [128, 128], BF16)
    trimask2 = din("trimask2", [128, 256], BF16)
    ones_row = din("ones_row", [1, 128], BF16)
    if n_layers:
        wqkv = din("wqkv", [n_layers, 128, KT, 3 * D], BF16)
        bqk = din("bqk", [n_layers, 128, 6], F32)
        wproj = din("wproj", [n_layers, 128, KT, D], BF16)
        bproj = din("bproj", [n_layers, 1, D], BF16)
        wff1 = din("wff1", [n_layers, 128, KT, FF], BF16)
        bff1 = din("bff1", [n_layers, 128, FT], F32)
        wff2 = din("wff2", [n_layers, 128, FT, D], BF16)
        bff2 = din("bff2", [n_layers, 1, D], BF16)
    whead = din("whead", [128, KT, V], BF16)
    bhead = din("bhead", [1, V], BF16)
    logits = nc.dram_tensor("logits", [NTOK, V], F32, kind="ExternalOutput").ap()
    dbg_spec = {
        "x0": (16, 128, D, F32), "xnT": (KT, 128, NTOK, BF16),
        "qT": (KT, 128, NTOK, BF16), "kT": (KT, 128, NTOK, BF16),
        "vt": (16, 128, D, BF16), "oT": (KT, 128, NTOK, BF16),
        "x1": (16, 128, D, F32), "x2": (16, 128, D, F32),
        "xnT2": (KT, 128, NTOK, BF16),
        "attE": (4, 128, 256, BF16), "attO": (4, 128, 256, F32),
    }
    dbg_ap = None
    if DBG is not None:
        n, p, c, dt = dbg_spec[DBG]
        dbg_ap = nc.dram_tensor("dbg", [n, p, c], dt, kind="ExternalOutput").ap()

    def dump(tag, tiles):
        if DBG == tag:
            for i, tl in enumerate(tiles):
                nc.sync.dma_start(out=dbg_ap[i], in_=tl)

    def dump3(tag, t3):
        if DBG == tag:
            for i in range(KT):
                nc.sync.dma_start(out=dbg_ap[i], in_=t3[:, i, :])

    with tile.TileContext(nc) as tc, ExitStack() as ctx:
        pool = lambda name, bufs: ctx.enter_context(tc.tile_pool(name=name, bufs=bufs))
        const_p = pool("const", 1)
        xres_p = pool("xres", 1)
        xnT_p = pool("xnT", 1)
        qkT_p = pool("qkT", 1)
        v_p = pool("vtok", 1)
        oT_p = pool("oT", 1)
        h_p = pool("hff", 1)
        w_p = pool("wts", 1)
        ln_p = pool("ln", 4)
        at_p = pool("attn", 4)
        cp_p = pool("cpy", 3)

        # ---- constants ----
        tri2_s = const_p.tile([128, 256], BF16)
        nc.sync.dma_start(out=tri2_s, in_=trimask2)
        ones_s = const_p.tile([1, 128], BF16)
        nc.sync.dma_start(out=ones_s, in_=ones_row)
        whead_s = const_p.tile([128, KT, V], BF16)
        nc.sync.dma_start(out=whead_s, in_=whead)
        bhead_s = const_p.tile([1, V], BF16)
        nc.sync.dma_start(out=bhead_s, in_=bhead)
        eps_t = const_p.tile([128, 1], F32)
        nc.vector.memset(eps_t, EPS)
        ones128 = const_p.tile([128, 128], BF16)
        nc.vector.memset(ones128, 1.0)

        # ---- persistent activation tiles ----
        x = [xres_p.tile([128, D], F32, tag=f"x{t}", name=f"x{t}") for t in range(TT)]
        xnT = xnT_p.tile([128, KT, NTOK], BF16, tag="xnT", name="xnT")
        qT = [qkT_p.tile([128, NTOK], BF16, tag=f"qT{k}", name=f"qT{k}") for k in range(KT)]
        kT = [qkT_p.tile([128, NTOK], BF16, tag=f"kT{k}", name=f"kT{k}") for k in range(KT)]
        vt = [v_p.tile([128, D], BF16, tag=f"v{t}", name=f"v{t}") for t in range(TT)]
        oT = oT_p.tile([128, KT, NTOK], BF16, tag="oT", name="oT")
        hh = h_p.tile([128, FT, FTOK], BF16, tag="hh", name="hh")

        # ---- embedding: x = onehot @ tok_emb + pos ----
        with tc.tile_pool(name="emb_ps", bufs=3, space="PSUM") as emb_ps, \
             tc.tile_pool(name="emb_sb", bufs=1) as emb_sb:
            oh_s = emb_sb.tile([V, NTOK], BF16)
            nc.sync.dma_start(out=oh_s, in_=onehotT)
            te_s = emb_sb.tile([V, D], BF16)
            nc.sync.dma_start(out=te_s, in_=tokemb)
            pos_s = emb_sb.tile([128, 2, D], BF16)
            nc.sync.dma_start(out=pos_s, in_=pos.rearrange("(a p) n -> p a n", p=128))
            id_s = emb_sb.tile([128, 128], BF16)
            nc.sync.dma_start(out=id_s, in_=ident)
            for t in range(TT):
                ps = emb_ps.tile([128, D], F32, tag="emb")
                nc.tensor.matmul(ps, lhsT=oh_s[:, t * 128:(t + 1) * 128],
                                 rhs=te_s, start=True, stop=False)
                nc.tensor.matmul(ps, lhsT=id_s, rhs=pos_s[:, t % 2, :],
                                 start=False, stop=True)
                nc.scalar.copy(out=x[t], in_=ps)
        dump("x0", x)

        def layernorm_and_transpose(dst_T):
            """token-major LN over x -> bf16 -> DMA-transpose into dst_T."""
            ctx2 = nc.named_scope("ln")
            ctx2.__enter__()
            for t in range(TT):
                stats = ln_p.tile([128, 6], F32, tag="stats")
                nc.vector.bn_stats(out=stats, in_=x[t])
                mv = ln_p.tile([128, 2], F32, tag="mv")
                nc.vector.bn_aggr(out=mv, in_=stats)
                rstd = ln_p.tile([128, 1], F32, tag="rstd")
                nc.scalar.activation(out=rstd, in_=mv[:, 1:2], func=AF.Sqrt,
                                     bias=eps_t)
                nc.vector.reciprocal(out=rstd, in_=rstd)
                nmr = ln_p.tile([128, 1], F32, tag="nmr")
                nc.vector.tensor_scalar(out=nmr, in0=mv[:, 0:1], scalar1=rstd,
                                        scalar2=-1.0, op0=OP.mult, op1=OP.mult)
                xn16 = ln_p.tile([128, D], BF16, tag="xn16")
                nc.scalar.activation(out=xn16, in_=x[t], func=AF.Identity,
                                     scale=rstd, bias=nmr)
                nc.sync.dma_start(out=dst_T[:, :, t * 128:(t + 1) * 128],
                                  in_=xn16, transpose=True)
            ctx2.__exit__(None, None, None)

        for l in range(n_layers):
            # ---- layer weights ----
            wqkv_s = w_p.tile([128, KT, 3 * D], BF16, tag="wqkv")
            nc.gpsimd.dma_start(out=wqkv_s, in_=wqkv[l])
            bqk_s = w_p.tile([128, 6], F32, tag="bqk")
            nc.gpsimd.dma_start(out=bqk_s, in_=bqk[l])
            wproj_s = w_p.tile([128, KT, D], BF16, tag="wproj")
            nc.gpsimd.dma_start(out=wproj_s, in_=wproj[l])
            bproj_s = w_p.tile([1, D], BF16, tag="bproj")
            nc.gpsimd.dma_start(out=bproj_s, in_=bproj[l])
            wff1_s = w_p.tile([128, KT, FF], BF16, tag="wff1")
            nc.gpsimd.dma_start(out=wff1_s, in_=wff1[l])
            bff1_s = w_p.tile([128, FT], F32, tag="bff1")
            nc.gpsimd.dma_start(out=bff1_s, in_=bff1[l])
            wff2_s = w_p.tile([128, FT, D], BF16, tag="wff2")
            nc.gpsimd.dma_start(out=wff2_s, in_=wff2[l])
            bff2_s = w_p.tile([1, D], BF16, tag="bff2")
            nc.gpsimd.dma_start(out=bff2_s, in_=bff2[l])

            # ---- LN1 -> xnT ----
            layernorm_and_transpose(xnT)
            if l == 0:
                dump3("xnT", xnT)

            # ---- qT, kT feature-major ----
            with tc.tile_pool(name="qk_ps", bufs=3, space="PSUM") as qk_ps, \
                 nc.named_scope("qkv"):
                for m in range(6):  # 6 chunks of 128 over q|k (768 cols)
                    dst = qT[m] if m < KT else kT[m - KT]
                    for n in range(NCH):
                        ns = slice(n * 512, (n + 1) * 512)
                        ps = qk_ps.tile([128, 512], F32, tag="qk")
                        for k in range(KT):
                            nc.tensor.matmul(
                                ps, lhsT=wqkv_s[:, k, m * 128:(m + 1) * 128],
                                rhs=xnT[:, k, ns], start=(k == 0), stop=(k == KT - 1))
                        nc.vector.tensor_scalar(out=dst[:, ns], in0=ps,
                                                scalar1=bqk_s[:, m:m + 1],
                                                scalar2=None, op0=OP.add)

                # ---- v token-major (same psum pool scope) ----
                # v bias is folded into proj bias host-side (o/l + bv).
                for t in range(TT):
                    ps = qk_ps.tile([128, D], F32, tag="vps")
                    for k in range(KT):
                        nc.tensor.matmul(ps, lhsT=xnT[:, k, t * 128:(t + 1) * 128],
                                         rhs=wqkv_s[:, k, 2 * D:3 * D],
                                         start=(k == 0), stop=(k == KT - 1))
                    nc.vector.tensor_copy(out=vt[t], in_=ps)
            if l == 0:
                dump("qT", qT)
                dump("kT", kT)
                dump("vt", vt)

            # ---- attention: head-pair batched, partition-packed l/o ----
            # Software-pipelined: scores for pair i+2 are issued before the
            # l/o matmuls of pair i, so the PE FIFO always has independent
            # work while pair i waits on its exp/mask chain.
            # Pair p covers heads (2p, 2p+1) = rows 0:64 / 64:128 of qT[p]/kT[p].
            # Per-head score tiles sit in SEPARATE psum banks (the two heads'
            # score MMs run on different PE row groups concurrently and must
            # not drain into one bank): layout [q0|q1 vs k0 (256) | q1k1 (128)]
            # l/o share one bank: cols 0:256 = l, 256:512 = o.
            with tc.tile_pool(name="sc_ps", bufs=2, space="PSUM") as sc_psp, \
                 tc.tile_pool(name="lo_ps", bufs=2, space="PSUM") as lo_psp, \
                 tc.tile_pool(name="prj_ps", bufs=2, space="PSUM") as prj_psp, \
                 nc.named_scope("attn"):
                pairs = [(b, p) for b in range(BPC) for p in range(H // 2)]
                LOOK = 2
                state = {}

                def emit_sc(b, p):
                    n0 = b * T
                    scs = [sc_psp.tile([128, 384], F32, tag="scA", name="scA"),
                           sc_psp.tile([128, 384], F32, tag="scB", name="scB")]
                    ehs = [at_p.tile([128, 384], BF16, tag="eA", name="eA"),
                           at_p.tile([128, 384], BF16, tag="eB", name="eB")]
                    for hh_ in range(2):
                        r = hh_ * 64
                        nc.tensor.matmul(
                            scs[hh_][:, 0:256],
                            lhsT=kT[p][r:r + 64, n0:n0 + 128],
                            rhs=qT[p][r:r + 64, n0:n0 + 256],
                            start=True, stop=True)
                        nc.tensor.matmul(
                            scs[hh_][:, 256:384],
                            lhsT=kT[p][r:r + 64, n0 + 128:n0 + 256],
                            rhs=qT[p][r:r + 64, n0 + 128:n0 + 256],
                            start=True, stop=True)
                        nc.scalar.activation(out=ehs[hh_], in_=scs[hh_],
                                             func=AF.Exp, scale=SCALE)
                        nc.vector.tensor_tensor(
                            out=ehs[hh_][:, 0:128], in0=ehs[hh_][:, 0:128],
                            in1=tri2_s[:, 0:128], op=OP.mult)
                        nc.vector.tensor_tensor(
                            out=ehs[hh_][:, 256:384],
                            in0=ehs[hh_][:, 256:384],
                            in1=tri2_s[:, 0:128], op=OP.mult)
                    state[(b, p)] = ehs

                def emit_tail(b, p):
                    n0 = b * T
                    ehs = state.pop((b, p))
                    lo = lo_psp.tile([128, 512], F32, tag="lo", name="lo")
                    l_ps, o_ps = lo[:, 0:256], lo[:, 256:512]
                    for hh_ in range(2):
                        r = hh_ * 64
                        tp = None if hh_ == 0 else (0, 64)
                        nc.tensor.matmul(
                            l_ps[r:r + 64, 0:256], lhsT=ones128[:, 0:64],
                            rhs=ehs[hh_][:, 0:256],
                            start=True, stop=False, tile_position=tp)
                        nc.tensor.matmul(
                            l_ps[r:r + 64, 128:256], lhsT=ones128[:, 0:64],
                            rhs=ehs[hh_][:, 256:384],
                            start=False, stop=True, tile_position=tp)
                        h = 2 * p + hh_
                        nc.tensor.matmul(
                            o_ps[r:r + 64, 0:256],
                            lhsT=vt[2 * b][:, h * 64:(h + 1) * 64],
                            rhs=ehs[hh_][:, 0:256],
                            start=True, stop=False, tile_position=tp)
                        nc.tensor.matmul(
                            o_ps[r:r + 64, 128:256],
                            lhsT=vt[2 * b + 1][:, h * 64:(h + 1) * 64],
                            rhs=ehs[hh_][:, 256:384],
                            start=False, stop=True, tile_position=tp)
                    linv = at_p.tile([128, 256], F32, tag="linv")
                    lnl = at_p.tile([128, 256], F32, tag="lnl")
                    nc.scalar.activation(out=lnl, in_=l_ps, func=AF.Ln)
                    nc.scalar.activation(out=linv, in_=lnl, func=AF.Exp,
                                         scale=-1.0)
                    nc.vector.tensor_tensor(out=oT[:, p, n0:n0 + 256],
                                            in0=o_ps, in1=linv, op=OP.mult)

                def emit_proj(b):
                    # proj for this batch's two token tiles; fills PE idle
                    # slots while later pairs wait on their exp/mask chain.
                    for t in (2 * b, 2 * b + 1):
                        ps = prj_psp.tile([128, D], F32, tag="tok", name="ptok")
                        for k in range(KT):
                            nc.tensor.matmul(
                                ps, lhsT=oT[:, k, t * 128:(t + 1) * 128],
                                rhs=wproj_s[:, k, :], start=(k == 0),
                                stop=(not with_bias and k == KT - 1))
                        if with_bias:
                            nc.tensor.matmul(ps, lhsT=ones_s,
                                             rhs=bproj_s, start=False, stop=True)
                        nc.vector.tensor_tensor(out=x[t], in0=x[t], in1=ps,
                                                op=OP.add)

                for i in range(len(pairs) + LOOK):
                    if i < len(pairs):
                        emit_sc(*pairs[i])
                    if i >= LOOK:
                        bb, pp = pairs[i - LOOK]
                        emit_tail(bb, pp)
                        if pp == H // 2 - 1:
                            emit_proj(bb)

            # ---- LN2 + FF (proj happened inside the attention loop) ----
            with tc.tile_pool(name="tok_ps", bufs=3, space="PSUM") as tok_ps, \
                 nc.named_scope("projff"):
                if l == 0:
                    dump("x1", x)

                # ---- LN2 -> xnT ----
                layernorm_and_transpose(xnT)
                if l == 0:
                    dump3("xnT2", xnT)

                # ---- FF in two 1024-token chunks ----
                for ch in range(FFCH):
                    c0 = ch * FTOK
                    for f in range(FT):
                        for n in range(FTOK // 512):
                            ns = slice(c0 + n * 512, c0 + (n + 1) * 512)
                            hs = slice(n * 512, (n + 1) * 512)
                            ps = tok_ps.tile([128, 512], F32, tag="ff1")
                            for k in range(KT):
                                nc.tensor.matmul(
                                    ps, lhsT=wff1_s[:, k, f * 128:(f + 1) * 128],
                                    rhs=xnT[:, k, ns], start=(k == 0),
                                    stop=(k == KT - 1))
                            nc.scalar.activation(out=hh[:, f, hs], in_=ps,
                                                 func=AF.Relu,
                                                 bias=bff1_s[:, f:f + 1])
                    for tt in range(FTOK // 128):
                        t = ch * (FTOK // 128) + tt
                        ps = tok_ps.tile([128, D], F32, tag="tok")
                        for f in range(FT):
                            nc.tensor.matmul(
                                ps, lhsT=hh[:, f, tt * 128:(tt + 1) * 128],
                                rhs=wff2_s[:, f, :], start=(f == 0),
                                stop=(not with_bias and f == FT - 1))
                        if with_bias:
                            nc.tensor.matmul(ps, lhsT=ones_s,
                                             rhs=bff2_s, start=False, stop=True)
                        nc.vector.tensor_tensor(out=x[t], in0=x[t], in1=ps, op=OP.add)
            if l == 0:
                dump("x2", x)

        # ---- final LN + head ----
        layernorm_and_transpose(xnT)
        with tc.tile_pool(name="hd_ps", bufs=3, space="PSUM") as hd_ps:
            for t in range(TT):
                ps = hd_ps.tile([128, V], F32, tag="hd")
                for k in range(KT):
                    nc.tensor.matmul(ps, lhsT=xnT[:, k, t * 128:(t + 1) * 128],
                                     rhs=whead_s[:, k, :], start=(k == 0),
                                     stop=(not with_bias and k == KT - 1))
                if with_bias:
                    nc.tensor.matmul(ps, lhsT=ones_s,
                                     rhs=bhead_s, start=False, stop=True)
                lt = cp_p.tile([128, V], F32, tag="logit")
                nc.scalar.copy(out=lt, in_=ps)
                nc.sync.dma_start(out=logits[t * 128:(t + 1) * 128, :], in_=lt)

    _split_multi_waits(nc)
    return nc


def prepare_host_inputs(idx, tok_emb, pos_emb, ln1_w, ln1_b, wq, wk, wv,
                        proj_w, proj_b, ln2_w, ln2_b, ff_w1, ff_b1, ff_w2,
                        ff_b2, lnf_w, lnf_b, head_w, head_b, n_layers=L):
    f32 = np.float32
    bf = ml_dtypes.bfloat16

    def kt_tiles(w, ncols):  # [D, ncols] -> [128, KT, ncols]
        return np.ascontiguousarray(
            w.reshape(KT, 128, ncols).transpose(1, 0, 2))

    wqkv_l, bqk_l = [], []
    wproj_l, bproj_l = [], []
    wff1_l, bff1_l, wff2_l, bff2_l = [], [], [], []
    for l in range(n_layers):
        # [H, D, HD] -> [D, H*HD]
        q2 = wq[l].transpose(1, 0, 2).reshape(D, D).astype(f32)
        k2 = wk[l].transpose(1, 0, 2).reshape(D, D).astype(f32)
        v2 = wv[l].transpose(1, 0, 2).reshape(D, D).astype(f32)
        qf = ln1_w[l][:, None] * q2
        kf = ln1_w[l][:, None] * k2
        vf = ln1_w[l][:, None] * v2
        bq = ln1_b[l] @ q2
        bk = ln1_b[l] @ k2
        bvv = ln1_b[l] @ v2
        wqkv_l.append(kt_tiles(np.concatenate([qf, kf, vf], axis=1), 3 * D))
        # col m (m<3) = bq[m*128+p]; col 3+m = bk[m*128+p]
        bqk_l.append(np.concatenate(
            [bq.reshape(KT, 128).T, bk.reshape(KT, 128).T], axis=1))
        wproj_l.append(kt_tiles(proj_w[l].astype(f32), D))
        # v bias folded through attention (o/l + bv) into the proj bias.
        bproj_l.append((proj_b[l] + bvv @ proj_w[l]).reshape(1, D))
        w1f = ln2_w[l][:, None] * ff_w1[l].astype(f32)
        b1f = ff_b1[l] + ln2_b[l] @ ff_w1[l]
        wff1_l.append(kt_tiles(w1f, FF))
        bff1_l.append(np.ascontiguousarray(
            b1f.reshape(FT, 128).T).astype(f32))
        wff2_l.append(np.ascontiguousarray(
            ff_w2[l].reshape(FT, 128, D).transpose(1, 0, 2)).astype(f32))
        bff2_l.append(ff_b2[l].reshape(1, D))

    whf = lnf_w[:, None] * head_w.astype(f32)
    bhf = head_b + lnf_b @ head_w

    def stk(lst, shape, dt):
        if lst:
            return np.stack(lst).astype(dt)
        return np.zeros((0,) + shape, dt)

    tri = np.triu(np.ones((128, 128), f32))  # [s,t] valid s<=t
    shared = {
        "tokemb": np.asarray(tok_emb).astype(bf),
        "pos": np.asarray(pos_emb).astype(bf),
        "ident": np.eye(128, dtype=np.float32).astype(bf),
        "trimask2": np.concatenate([tri, tri], axis=1).astype(bf),
        "ones_row": np.ones((1, 128), bf),
        "wqkv": stk(wqkv_l, (128, KT, 3 * D), bf),
        "bqk": stk(bqk_l, (128, 6), f32),
        "wproj": stk(wproj_l, (128, KT, D), bf),
        "bproj": stk(bproj_l, (1, D), bf),
        "wff1": stk(wff1_l, (128, KT, FF), bf),
        "bff1": stk(bff1_l, (128, FT), f32),
        "wff2": stk(wff2_l, (128, FT, D), bf),
        "bff2": stk(bff2_l, (1, D), bf),
        "whead": kt_tiles(whf, V).astype(bf),
        "bhead": bhf.reshape(1, V).astype(bf),
    }
    idx = np.asarray(idx)
    in_maps = []
    shared["_zero_bias"] = (
        not np.any(proj_b) and not np.any(ff_b2) and not np.any(head_b)
        and not np.any(ln1_b @ np.stack([wv[l].transpose(1, 0, 2).reshape(D, D)
                                         for l in range(n_layers)], 0)
                       if n_layers else 0))
    for c in range(NCORES):
        oh = np.zeros((V, NTOK), bf)
        sl = idx[c * BPC:(c + 1) * BPC].reshape(NTOK)  # token n = b*T + t
        oh[sl, np.arange(NTOK)] = 1.0
        in_maps.append({**shared, "onehotT": oh})
    return in_maps


_CACHED = {}


def kernel(**inputs):
    n_layers = L
    in_maps = prepare_host_inputs(**inputs, n_layers=n_layers)
    zb = all(m.pop("_zero_bias", False) for m in in_maps)
    key = ("nc", zb)
    if key not in _CACHED:
        _CACHED[key] = build(n_layers, with_bias=not zb)
    nc = _CACHED[key]
    res = run_bass_kernel_spmd(nc, in_maps, list(range(NCORES)))
    out = np.concatenate(
        [res.results[c]["logits"].reshape(BPC, T, V) for c in range(NCORES)],
        axis=0)
    return out

